# revision 17
# baseline (speedup 1.0000x reference)
"""Trainium2 Bass kernel: pointer-generator decoder step (nn_Decoder).

Data-parallel over batch B=256 across 8 NeuronCores (32 rows/core). Host
pre-transposes/casts operands into matmul-friendly layouts and pre-computes
the scatter structure (segment/one-hot matrices) from the input index
tensor. Device computes: x-linear, LSTM cell, encoder intra-temporal
attention (the dominant [B*N,2H]x[2H,2H] matmul), decoder intra-attention,
pointer-generator head, 50k-vocab projection + softmax, and the scatter of
attention mass: per-column segment matmuls combine same-destination-row
contributions into 64-float rows, GPSIMD dma_scatter_add (overwrite
semantics on this stack) places them into a zero-initialized DRAM scratch
(one padded 783-row span per batch row so rows never alias across b), and a
dense final = base + (1-p_gen)*scratch pass produces the output.

The encoder n-axis is permuted per row on the host (applied consistently to
every n-indexed input; sum_new is un-permuted on output) so that scatter
destination groups are contiguous and bin-packable into whole columns.
"""

import numpy as np
import ml_dtypes

import concourse.bass as bass
import concourse.tile as tile
from concourse import bacc, mybir
from concourse.bass_utils import run_bass_kernel_spmd
from concourse.masks import make_identity
import bass_rust

B, N, H, E, V, OOV, T = 256, 400, 512, 256, 50000, 50, 49
NCORES = 8
BL = B // NCORES          # 32 rows per core
H2 = 2 * H                # 1024
VO = V + OOV              # 50050
QCOLS = 4 * BL            # 128 real scatter columns (4 per batch row)
NIDX = QCOLS * 128        # scatter slots (128 per column)
RPB = (VO + 63) // 64 + 1 # 783 scratch rows per batch row (no aliasing)
AROWS = 128 * ((BL * RPB + 1 + 127) // 128)  # padded scratch row count
SAFE = BL * RPB           # safe dummy row
TP = 64                   # padded T

BF = mybir.dt.bfloat16
F32 = mybir.dt.float32
I16 = mybir.dt.int16
bf16 = ml_dtypes.bfloat16

AF = mybir.ActivationFunctionType
ALU = mybir.AluOpType


# ---------------------------------------------------------------- host prep

def _to_tiles(mat, kt):
    nf = mat.shape[1]
    return np.ascontiguousarray(mat.reshape(kt, 128, nf).transpose(1, 0, 2))


def _pack_rhs(blocks, bias):
    mat = np.concatenate(blocks, 0)
    k, nf = mat.shape
    assert k % 128 == 0
    kt = k // 128 + 1
    out = np.zeros((kt * 128, nf), np.float32)
    out[:k] = mat
    out[k] = bias
    return _to_tiles(out, kt).astype(bf16)


def _packT(x, kt=None, ones_row=False):
    xt = x.T
    c = xt.shape[0]
    if kt is None:
        kt = (c + (1 if ones_row else 0) + 127) // 128
    out = np.zeros((kt * 128, xt.shape[1]), np.float32)
    out[:c] = xt
    if ones_row:
        out[c] = 1.0
    return _to_tiles(out, kt).astype(bf16)


def _scatter_struct(idx_rows):
    """idx_rows: [BL, N] raw destination columns. Computes the per-row
    permutation (sort by destination, then pack same-64-block groups into 4
    columns with EXACT quotas 128/128/128/16 so the compact layout is
    uniform across rows and cores) plus the device scatter structure.
    Returns (perm [BL, N], seg, oh, idxs)."""
    QUOTA = (128, 128, 128, 16)
    perm = np.empty((BL, N), np.int64)
    seg = np.zeros((128, QCOLS, 128), np.float32)
    oh = np.zeros((128, QCOLS, 64), np.float32)
    idxs_flat = np.full(NIDX, SAFE, np.int64)
    for b in range(BL):
        order = np.argsort(idx_rows[b], kind="stable")
        sidx = idx_rows[b][order]
        rows = sidx // 64
        bounds = [0] + list(np.nonzero(rows[1:] != rows[:-1])[0] + 1) + [N]
        groups = [(bounds[i], bounds[i + 1]) for i in range(len(bounds) - 1)]
        # first-fit decreasing into exact quotas (caps sum == N)
        cap = list(QUOTA)
        colgrp = [[], [], [], []]
        for gi in sorted(range(len(groups)),
                         key=lambda i: groups[i][1] - groups[i][0],
                         reverse=True):
            lo, hi = groups[gi]
            gl = hi - lo
            assert gl <= 128, "destination 64-block group too large"
            for q in range(4):
                if cap[q] >= gl:
                    cap[q] -= gl
                    colgrp[q].append(gi)
                    break
            else:
                raise AssertionError("quota packing failed")
        assert cap == [0, 0, 0, 0]
        for q in range(4):
            qg = 4 * b + q
            p = 0
            for m, gi in enumerate(sorted(colgrp[q])):
                lo, hi = groups[gi]
                idxs_flat[qg * 128 + m] = b * RPB + int(rows[lo])
                for src in range(lo, hi):
                    perm[b, q * 128 + p] = order[src]
                    seg[p, qg, m] = 1.0
                    oh[p, qg, sidx[src] % 64] = 1.0
                    p += 1
            assert p == QUOTA[q]
    idxs = np.zeros((16, NIDX // 16), np.int16)
    for i in range(NIDX):
        idxs[i % 16, i // 16] = idxs_flat[i]
    idxs = np.tile(idxs, (8, 1))          # replicated across the 8 Q7 cores
    return perm, seg.astype(bf16), oh.astype(bf16), idxs


def prep_inputs(inputs):
    g = {k: np.asarray(v) for k, v in inputs.items()}
    f32 = lambda k: np.asarray(g[k], np.float32)

    shared = {}
    shared["xiW"] = _pack_rhs([f32("x_input_w").T], f32("x_input_b"))
    shared["gatesW"] = _pack_rhs(
        [f32("lstm_w_ih").T, f32("lstm_w_hh").T],
        f32("lstm_b_ih") + f32("lstm_b_hh"))
    shared["wsT"] = _pack_rhs([f32("Ws_w").T], f32("Ws_b"))
    shared["wsdT"] = _pack_rhs([f32("Wsd_w").T], f32("Wsd_b"))
    shared["vwT"] = _pack_rhs([f32("V_w").T], f32("V_b"))
    shared["pgenW"] = _pack_rhs([f32("pgen_w").T], f32("pgen_b"))
    shared["whT"] = _to_tiles(f32("Wh_w").T, 8).astype(bf16)
    shared["wprevT"] = _to_tiles(f32("Wprev_w").T, 4).astype(bf16)
    shared["v1T"] = _to_tiles(f32("V1_w").T, 4).astype(bf16)
    shared["vv"] = _to_tiles(f32("v_w").T, 8).astype(bf16)
    shared["vdv"] = _to_tiles(f32("vd_w").T, 4).astype(bf16)
    v1b = f32("V1_b")
    v1_bias_nonzero = bool(np.any(v1b))
    if v1_bias_nonzero:
        shared["v1b"] = np.ascontiguousarray(v1b[None, :]).astype(bf16)

    enc = f32("enc_out")
    prev = f32("prev_s")
    xct = np.concatenate([f32("x_t"), f32("ct_e")], axis=1)
    mask = f32("enc_padding_mask")
    sumt = f32("sum_temporal_srcs")
    minv = mask / sumt
    idx_all = np.asarray(g["enc_batch_extend_vocab"]).astype(np.int64)
    ez = f32("extra_zeros")

    in_maps = []
    perms = []
    for c in range(NCORES):
        s = slice(c * BL, (c + 1) * BL)
        m = dict(shared)
        perm, seg, oh, idxs = _scatter_struct(idx_all[s])
        perms.append(perm)
        encp = np.take_along_axis(enc[s], perm[:, :, None], axis=1)

        m["encT"] = np.ascontiguousarray(
            encp.transpose(2, 0, 1).reshape(8, 128, BL * N)
            .transpose(1, 0, 2)).astype(bf16)
        encnat = np.zeros((BL, 4, 128, H2), np.float32)
        encnat.reshape(BL, 512, H2)[:, :N] = encp
        m["encnat"] = encnat.astype(bf16)
        m["prevT"] = np.ascontiguousarray(
            prev[s].transpose(2, 0, 1).reshape(4, 128, BL * T)
            .transpose(1, 0, 2)).astype(bf16)
        prevnat = np.zeros((BL, TP, H), np.float32)
        prevnat[:, :T] = prev[s]
        m["prevnat"] = prevnat.astype(bf16)
        m["xctT"] = _packT(xct[s], kt=11, ones_row=True)
        m["dechT"] = _packT(f32("dec_h")[s], kt=4)
        m["dec_c"] = np.ascontiguousarray(f32("dec_c")[s])
        m["minv"] = np.ascontiguousarray(
            np.take_along_axis(minv[s], perm, axis=1))
        m["sum_t"] = np.ascontiguousarray(
            np.take_along_axis(sumt[s], perm, axis=1))
        m["ez"] = np.ascontiguousarray(ez[s])
        m["seg"] = seg
        m["oh"] = oh
        m["idxs"] = idxs
        in_maps.append(m)

    return in_maps, v1_bias_nonzero, perms


# ------------------------------------------------------------ device build

def build_program(v1_bias_nonzero):
    nc = bacc.Bacc("TRN2", target_bir_lowering=False, debug=False)
    D = {}

    def din(name, shape, dt=BF):
        D[name] = nc.dram_tensor(name, list(shape), dt, kind="ExternalInput").ap()

    def dout(name, shape, dt=F32):
        D[name] = nc.dram_tensor(name, list(shape), dt, kind="ExternalOutput").ap()

    din("xiW", [128, 11, 256]); din("gatesW", [128, 7, 2048])
    din("wsT", [128, 9, 1024]); din("wsdT", [128, 5, 512])
    din("vwT", [128, 17, 512]); din("pgenW", [128, 23, 1])
    din("whT", [128, 8, 1024]); din("wprevT", [128, 4, 512])
    din("v1T", [128, 4, V]); din("vv", [128, 8, 1]); din("vdv", [128, 4, 1])
    if v1_bias_nonzero:
        din("v1b", [1, V])
    din("encT", [128, 8, BL * N]); din("encnat", [BL, 4, 128, H2])
    din("prevT", [128, 4, BL * T]); din("prevnat", [BL, TP, H])
    din("xctT", [128, 11, BL]); din("dechT", [128, 4, BL])
    din("dec_c", [BL, H], F32)
    din("minv", [BL, N], F32); din("sum_t", [BL, N], F32)
    din("ez", [BL, OOV], F32)
    din("seg", [128, QCOLS, 128]); din("oh", [128, QCOLS, 64])
    din("idxs", [128, NIDX // 16], I16)

    adds_t = nc.dram_tensor("adds", [AROWS, 64], F32)
    at_dram = nc.dram_tensor("at_dram", [BL, 512], F32)

    dout("final", [BL, VO]); dout("h_new", [BL, H]); dout("c_new", [BL, H])
    dout("ct_e", [BL, H2]); dout("sum_new", [BL, N])

    with tile.TileContext(nc) as tc:
        _body(nc, tc, D, adds_t, at_dram, v1_bias_nonzero)

    nc.compile()
    return nc


def _mm_affine(nc, out_ps, lhs_tiles, rhs_sb, nf_slice, ones_tile):
    kt = len(lhs_tiles) + 1
    for k in range(kt):
        lhs = lhs_tiles[k] if k < len(lhs_tiles) else ones_tile
        nc.tensor.matmul(out_ps, lhs, rhs_sb[:, k, nf_slice],
                         start=(k == 0), stop=(k == kt - 1))


def _body(nc, tc, D, adds_t, at_dram, v1_bias_nonzero):
    from contextlib import ExitStack

    with ExitStack() as ctx:
        const = ctx.enter_context(tc.tile_pool(name="const", bufs=1))
        persist = ctx.enter_context(tc.tile_pool(name="persist", bufs=1))

        ident_bf = const.tile([128, 128], BF)
        make_identity(nc, ident_bf[:])
        ident_f = const.tile([128, 128], F32)
        make_identity(nc, ident_f[:])
        ones_bf = const.tile([128, BL], BF)       # row0 = 1 (bias K-tile)
        nc.gpsimd.memset(ones_bf[:], 0.0)
        nc.gpsimd.memset(ones_bf[:1, :], 1.0)
        ones_col = const.tile([128, 1], BF)       # all-ones (reduce lhsT)
        nc.gpsimd.memset(ones_col[:], 1.0)

        xT = persist.tile([128, 2, BL], BF)
        dechT = persist.tile([128, 4, BL], BF)
        nc.sync.dma_start(dechT[:], D["dechT"][:])
        hT = persist.tile([128, 4, BL], BF)
        stT = persist.tile([128, 8, BL], BF)
        sprojT = persist.tile([128, 8, BL], F32)
        hdprojT = persist.tile([128, 4, BL], F32)
        ct_eT_bf = persist.tile([128, 8, BL], BF)
        ct_dT_bf = persist.tile([128, 4, BL], BF)
        vv_sb = const.tile([128, 8, 1], BF)
        nc.sync.dma_start(vv_sb[:], D["vv"][:])
        vdv_sb = const.tile([128, 4, 1], BF)
        nc.sync.dma_start(vdv_sb[:], D["vdv"][:])
        exp_all = persist.tile([BL, N], F32)
        ct_e_sb = persist.tile([BL, H2], F32)
        ct_d_sb = persist.tile([BL, H], F32)
        v_items = persist.tile([128, QCOLS], F32)
        pg_col = persist.tile([BL, 1], F32)
        ompg = persist.tile([BL, 1], F32)

        # ---------------- stage 1 ----------------------------------------
        with tc.tile_pool(name="s1w", bufs=1) as s1w, \
             tc.tile_pool(name="s1", bufs=1) as s1, \
             tc.tile_pool(name="s1ps", bufs=1, space="PSUM") as s1ps, \
             tc.tile_pool(name="s1tp", bufs=2, space="PSUM") as tp_ps:
            xiW = s1w.tile([128, 11, 256], BF)
            nc.sync.dma_start(xiW[:], D["xiW"][:])
            gatesW = s1w.tile([128, 7, 2048], BF)
            nc.sync.dma_start(gatesW[:], D["gatesW"][:])
            wsT = s1w.tile([128, 9, 1024], BF)
            nc.sync.dma_start(wsT[:], D["wsT"][:])
            wsdT = s1w.tile([128, 5, 512], BF)
            nc.sync.dma_start(wsdT[:], D["wsdT"][:])
            xctT = s1w.tile([128, 11, BL], BF)
            nc.sync.dma_start(xctT[:], D["xctT"][:])

            x_ps = s1ps.tile([BL, 256], F32, tag="big")
            for k in range(11):
                nc.tensor.matmul(x_ps[:], xctT[:, k, :], xiW[:, k, :],
                                 start=(k == 0), stop=(k == 10))
            x_bf = s1.tile([BL, 256], BF)
            nc.vector.tensor_copy(x_bf[:], x_ps[:])
            for k in range(2):
                ps = tp_ps.tile([128, BL], BF, tag="tp")
                nc.tensor.transpose(ps[:], x_bf[:, k * 128:(k + 1) * 128],
                                    ident_bf[:BL, :BL])
                nc.vector.tensor_copy(xT[:, k, :], ps[:])

            lhs = [xT[:, k, :] for k in range(2)] + \
                  [dechT[:, k, :] for k in range(4)]
            gates_ps = s1ps.tile([BL, 2048], F32, tag="big")
            for j in range(4):
                _mm_affine(nc, gates_ps[:, j * 512:(j + 1) * 512], lhs, gatesW,
                           slice(j * 512, (j + 1) * 512), ones_bf[:])
            sig_i = s1.tile([BL, H], F32)
            nc.scalar.activation(sig_i[:], gates_ps[:, 0:H], AF.Sigmoid)
            sig_f = s1.tile([BL, H], F32)
            nc.scalar.activation(sig_f[:], gates_ps[:, H:2 * H], AF.Sigmoid)
            tanh_g = s1.tile([BL, H], F32)
            nc.scalar.activation(tanh_g[:], gates_ps[:, 2 * H:3 * H], AF.Tanh)
            sig_o = s1.tile([BL, H], F32)
            nc.scalar.activation(sig_o[:], gates_ps[:, 3 * H:4 * H], AF.Sigmoid)
            dec_c = s1.tile([BL, H], F32)
            nc.sync.dma_start(dec_c[:], D["dec_c"][:])
            t1 = s1.tile([BL, H], F32)
            nc.vector.tensor_tensor(t1[:], sig_f[:], dec_c[:], ALU.mult)
            t2 = s1.tile([BL, H], F32)
            nc.vector.tensor_tensor(t2[:], sig_i[:], tanh_g[:], ALU.mult)
            c_new = s1.tile([BL, H], F32)
            nc.vector.tensor_tensor(c_new[:], t1[:], t2[:], ALU.add)
            tanh_c = s1.tile([BL, H], F32)
            nc.scalar.activation(tanh_c[:], c_new[:], AF.Tanh)
            h_new = s1.tile([BL, H], F32)
            nc.vector.tensor_tensor(h_new[:], sig_o[:], tanh_c[:], ALU.mult)
            nc.sync.dma_start(D["h_new"][:], h_new[:])
            nc.sync.dma_start(D["c_new"][:], c_new[:])

            st_bf = s1.tile([BL, H2], BF)
            nc.vector.tensor_copy(st_bf[:, :H], h_new[:])
            nc.vector.tensor_copy(st_bf[:, H:], c_new[:])
            for k in range(8):
                ps = tp_ps.tile([128, BL], BF, tag="tp")
                nc.tensor.transpose(ps[:], st_bf[:, k * 128:(k + 1) * 128],
                                    ident_bf[:BL, :BL])
                nc.vector.tensor_copy(stT[:, k, :], ps[:])
            for k in range(4):
                nc.vector.tensor_copy(hT[:, k, :], stT[:, k, :])

            st_lhs = [stT[:, k, :] for k in range(8)]
            for j in range(2):
                sp_ps = s1ps.tile([BL, 512], F32, tag="big")
                _mm_affine(nc, sp_ps[:], st_lhs, wsT,
                           slice(j * 512, (j + 1) * 512), ones_bf[:])
                sp_sb = s1.tile([BL, 512], F32, tag="spsb")
                nc.vector.tensor_copy(sp_sb[:], sp_ps[:])
                for k in range(4):
                    ps = tp_ps.tile([128, BL], F32, tag="tpf")
                    nc.tensor.transpose(ps[:], sp_sb[:, k * 128:(k + 1) * 128],
                                        ident_f[:BL, :BL])
                    nc.vector.tensor_copy(sprojT[:, j * 4 + k, :], ps[:])

            h_lhs = [hT[:, k, :] for k in range(4)]
            hd_ps = s1ps.tile([BL, 512], F32, tag="big")
            _mm_affine(nc, hd_ps[:], h_lhs, wsdT, slice(0, 512), ones_bf[:])
            hd_sb = s1.tile([BL, 512], F32, tag="spsb")
            nc.vector.tensor_copy(hd_sb[:], hd_ps[:])
            for k in range(4):
                ps = tp_ps.tile([128, BL], F32, tag="tpf")
                nc.tensor.transpose(ps[:], hd_sb[:, k * 128:(k + 1) * 128],
                                    ident_f[:BL, :BL])
                nc.vector.tensor_copy(hdprojT[:, k, :], ps[:])

        # ---------------- stage 2: encoder attention ----------------------
        with tc.tile_pool(name="s2w", bufs=1) as s2w, \
             tc.tile_pool(name="s2", bufs=2) as s2, \
             tc.tile_pool(name="s2row", bufs=2) as s2row, \
             tc.tile_pool(name="zps", bufs=2, space="PSUM") as zps, \
             tc.tile_pool(name="etps", bufs=1, space="PSUM") as etps, \
             tc.tile_pool(name="ctps", bufs=1, space="PSUM") as ctps, \
             tc.tile_pool(name="tp2", bufs=1, space="PSUM") as tp2:
            whT = s2w.tile([128, 8, 1024], BF)
            nc.sync.dma_start(whT[:], D["whT"][:])

            for b in range(BL):
                encb = s2.tile([128, 8, N], BF, tag="encb")
                nc.sync.dma_start(encb[:], D["encT"][:, :, b * N:(b + 1) * N])
                tz = s2.tile([128, 8, N], BF, tag="tz")
                for half in range(4):
                    zt = zps.tile([128, 2, 512], F32, tag="z")
                    for j in range(2):
                        em = half * 2 + j
                        for k in range(8):
                            nc.tensor.matmul(
                                zt[:, j, :N], whT[:, k, em * 128:(em + 1) * 128],
                                encb[:, k, :], start=(k == 0), stop=(k == 7))
                        nc.scalar.activation(tz[:, em, :], zt[:, j, :N], AF.Tanh,
                                             bias=sprojT[:, em, b:b + 1])
                et_ps = etps.tile([1, N], F32, tag="et")
                for k in range(8):
                    nc.tensor.matmul(et_ps[:], vv_sb[:, k, :], tz[:, k, :],
                                     start=(k == 0), stop=(k == 7))
                exp_row = s2row.tile([1, N], F32, tag="exp")
                nc.scalar.activation(exp_row[:], et_ps[:], AF.Exp)
                nc.sync.dma_start(exp_all[b:b + 1, :], exp_row[:])
                minv_row = s2row.tile([1, N], F32, tag="minv")
                nc.sync.dma_start(minv_row[:], D["minv"][b:b + 1, :])
                w_row = s2row.tile([1, N], F32, tag="w")
                nc.vector.tensor_tensor(w_row[:], exp_row[:], minv_row[:],
                                        ALU.mult)
                zsum = s2row.tile([1, 1], F32, tag="zsum")
                nc.vector.tensor_reduce(zsum[:], w_row[:],
                                        mybir.AxisListType.X, ALU.add)
                zinv = s2row.tile([1, 1], F32, tag="zinv")
                nc.vector.reciprocal(zinv[:], zsum[:])
                at_row = s2row.tile([1, 512], F32, tag="at")
                nc.vector.memset(at_row[:], 0.0)
                nc.vector.tensor_scalar(at_row[:, :N], w_row[:], zinv[:],
                                        None, ALU.mult)
                # at values -> DRAM bounce (gathered into v_items later)
                nc.sync.dma_start(at_dram.ap()[b:b + 1, :], at_row[:])
                # at as per-partition columns for the ct_e weighted reduce
                at_colT = s2row.tile([128, 4], F32, tag="atc")
                for k in range(4):
                    ps = tp2.tile([128, 1], F32, tag="tpa")
                    nc.tensor.transpose(ps[:],
                                        at_row[:1, k * 128:(k + 1) * 128],
                                        ident_f[:1, :1])
                    nc.vector.tensor_copy(at_colT[:, k:k + 1], ps[:])
                encnb = s2.tile([128, 4, H2], BF, tag="encnb")
                nc.sync.dma_start(encnb[:],
                                  D["encnat"][b].rearrange("k p d -> p k d"))
                ct_ps = ctps.tile([1, H2], F32, tag="ct")
                for k in range(4):
                    prod = s2.tile([128, H2], BF, tag="prod")
                    nc.vector.tensor_scalar(prod[:], encnb[:, k, :],
                                            at_colT[:, k:k + 1], None, ALU.mult)
                    for j in range(2):
                        nc.tensor.matmul(
                            ct_ps[:, j * 512:(j + 1) * 512], ones_col[:],
                            prod[:, j * 512:(j + 1) * 512],
                            start=(k == 0), stop=(k == 3))
                ct_row = s2row.tile([1, H2], F32, tag="ctr")
                nc.vector.tensor_copy(ct_row[:], ct_ps[:])
                nc.sync.dma_start(ct_e_sb[b:b + 1, :], ct_row[:])

        nc.sync.dma_start(D["ct_e"][:], ct_e_sb[:])

        with tc.tile_pool(name="sn", bufs=1) as sn:
            sum_t = sn.tile([BL, N], F32)
            nc.sync.dma_start(sum_t[:], D["sum_t"][:])
            sum_new = sn.tile([BL, N], F32)
            nc.vector.tensor_tensor(sum_new[:], sum_t[:], exp_all[:], ALU.add)
            nc.sync.dma_start(D["sum_new"][:], sum_new[:])

        with tc.tile_pool(name="cte", bufs=1) as cte, \
             tc.tile_pool(name="ctetp", bufs=2, space="PSUM") as ctetp:
            ce_bf = cte.tile([BL, H2], BF)
            nc.vector.tensor_copy(ce_bf[:], ct_e_sb[:])
            for k in range(8):
                ps = ctetp.tile([128, BL], BF, tag="tp")
                nc.tensor.transpose(ps[:], ce_bf[:, k * 128:(k + 1) * 128],
                                    ident_bf[:BL, :BL])
                nc.vector.tensor_copy(ct_eT_bf[:, k, :], ps[:])

        # ---------------- stage 3: decoder intra-attention ----------------
        with tc.tile_pool(name="s3w", bufs=1) as s3w, \
             tc.tile_pool(name="s3", bufs=2) as s3, \
             tc.tile_pool(name="s3row", bufs=2) as s3row, \
             tc.tile_pool(name="s3zd", bufs=1, space="PSUM") as s3zd, \
             tc.tile_pool(name="s3ps", bufs=1, space="PSUM") as s3ps, \
             tc.tile_pool(name="s3tp", bufs=1, space="PSUM") as s3tp:
            wprevT = s3w.tile([128, 4, 512], BF)
            nc.sync.dma_start(wprevT[:], D["wprevT"][:])
            prevT = s3w.tile([128, 4, BL * T], BF)
            nc.sync.dma_start(prevT[:], D["prevT"][:])

            for b in range(BL):
                pb = prevT[:, :, b * T:(b + 1) * T]
                zd = s3zd.tile([128, 4, 512], F32, tag="zd")
                tzd = s3.tile([128, 4, T], BF, tag="tzd")
                for gq in range(4):
                    for k in range(4):
                        nc.tensor.matmul(
                            zd[:, gq, :T], wprevT[:, k, gq * 128:(gq + 1) * 128],
                            pb[:, k, :], start=(k == 0), stop=(k == 3))
                    nc.scalar.activation(tzd[:, gq, :], zd[:, gq, :T], AF.Tanh,
                                         bias=hdprojT[:, gq, b:b + 1])
                etd_ps = s3ps.tile([1, T], F32, tag="etd")
                for k in range(4):
                    nc.tensor.matmul(etd_ps[:], vdv_sb[:, k, :], tzd[:, k, :],
                                     start=(k == 0), stop=(k == 3))
                expd = s3row.tile([1, T], F32, tag="expd")
                zd_sum = s3row.tile([1, 1], F32, tag="zdsum")
                nc.scalar.activation(expd[:], etd_ps[:], AF.Exp,
                                     accum_out=zd_sum[:])
                zd_inv = s3row.tile([1, 1], F32, tag="zdinv")
                nc.vector.reciprocal(zd_inv[:], zd_sum[:])
                atd_row = s3row.tile([1, T], F32, tag="atd")
                nc.vector.tensor_scalar(atd_row[:], expd[:], zd_inv[:],
                                        None, ALU.mult)
                atd_ps = s3tp.tile([TP, 1], F32, tag="tpd")
                nc.tensor.transpose(atd_ps[:T, :], atd_row[:1, :],
                                    ident_f[:1, :1])
                atd_col = s3row.tile([TP, 1], F32, tag="atdc")
                nc.vector.memset(atd_col[:], 0.0)
                nc.vector.tensor_copy(atd_col[:T, :], atd_ps[:T, :])
                pnb = s3.tile([TP, H], BF, tag="pnb")
                nc.sync.dma_start(pnb[:], D["prevnat"][b])
                prodd = s3.tile([TP, H], BF, tag="prodd")
                nc.vector.tensor_scalar(prodd[:], pnb[:], atd_col[:],
                                        None, ALU.mult)
                ctd_ps = s3ps.tile([1, H], F32, tag="ctd")
                nc.tensor.matmul(ctd_ps[:], ones_col[:TP, :], prodd[:],
                                 start=True, stop=True)
                ctd_row = s3row.tile([1, H], F32, tag="ctdr")
                nc.vector.tensor_copy(ctd_row[:], ctd_ps[:])
                nc.sync.dma_start(ct_d_sb[b:b + 1, :], ctd_row[:])

        with tc.tile_pool(name="ctd", bufs=1) as ctdp, \
             tc.tile_pool(name="ctdtp", bufs=2, space="PSUM") as ctdtp:
            cd_bf = ctdp.tile([BL, H], BF)
            nc.vector.tensor_copy(cd_bf[:], ct_d_sb[:])
            for k in range(4):
                ps = ctdtp.tile([128, BL], BF, tag="tp")
                nc.tensor.transpose(ps[:], cd_bf[:, k * 128:(k + 1) * 128],
                                    ident_bf[:BL, :BL])
                nc.vector.tensor_copy(ct_dT_bf[:, k, :], ps[:])

        # ---------------- stage 4a: p_gen, out-vec, vocab ------------------
        vocab_pool = ctx.enter_context(tc.tile_pool(name="vocab", bufs=1))
        vocab_bf = vocab_pool.tile([BL, V], BF)

        with tc.tile_pool(name="s4w", bufs=1) as s4w, \
             tc.tile_pool(name="s4", bufs=1) as s4, \
             tc.tile_pool(name="v1s", bufs=3) as v1s, \
             tc.tile_pool(name="s4ps", bufs=1, space="PSUM") as s4ps, \
             tc.tile_pool(name="lps", bufs=3, space="PSUM") as lps, \
             tc.tile_pool(name="s4tp", bufs=2, space="PSUM") as tp_ps:
            vwT = s4w.tile([128, 17, 512], BF)
            nc.sync.dma_start(vwT[:], D["vwT"][:])
            pgenW = s4w.tile([128, 23, 1], BF)
            nc.sync.dma_start(pgenW[:], D["pgenW"][:])

            pg_lhs = ([ct_eT_bf[:, k, :] for k in range(8)]
                      + [ct_dT_bf[:, k, :] for k in range(4)]
                      + [stT[:, k, :] for k in range(8)]
                      + [xT[:, k, :] for k in range(2)])
            pg_ps = s4ps.tile([BL, 1], F32, tag="pg")
            _mm_affine(nc, pg_ps[:], pg_lhs, pgenW, slice(0, 1), ones_bf[:])
            nc.scalar.activation(pg_col[:], pg_ps[:], AF.Sigmoid)
            nc.vector.tensor_scalar(ompg[:], pg_col[:], -1.0, 1.0, ALU.mult,
                                    ALU.add)

            ov_lhs = ([hT[:, k, :] for k in range(4)]
                      + [ct_eT_bf[:, k, :] for k in range(8)]
                      + [ct_dT_bf[:, k, :] for k in range(4)])
            ov_ps = s4ps.tile([BL, 512], F32, tag="ov")
            _mm_affine(nc, ov_ps[:], ov_lhs, vwT, slice(0, 512), ones_bf[:])
            ov_bf = s4.tile([BL, 512], BF)
            nc.vector.tensor_copy(ov_bf[:], ov_ps[:])
            ovT = s4.tile([128, 4, BL], BF)
            for k in range(4):
                ps = tp_ps.tile([128, BL], BF, tag="tp")
                nc.tensor.transpose(ps[:], ov_bf[:, k * 128:(k + 1) * 128],
                                    ident_bf[:BL, :BL])
                nc.vector.tensor_copy(ovT[:, k, :], ps[:])

            NCH = (V + 511) // 512
            psums = s4.tile([BL, 128], F32)
            if v1_bias_nonzero:
                v1b_sb = s4w.tile([1, V], BF)
                nc.sync.dma_start(v1b_sb[:], D["v1b"][:])
            for cix in range(NCH):
                lo = cix * 512
                hi = min(lo + 512, V)
                w = hi - lo
                v1c = v1s.tile([128, 4, 512], BF, tag="v1c")
                nc.sync.dma_start(v1c[:, :, :w], D["v1T"][:, :, lo:hi])
                lp = lps.tile([BL, 512], F32, tag="lg")
                nkt = 5 if v1_bias_nonzero else 4
                for k in range(4):
                    nc.tensor.matmul(lp[:, :w], ovT[:, k, :], v1c[:, k, :w],
                                     start=(k == 0), stop=(k == nkt - 1))
                if v1_bias_nonzero:
                    nc.tensor.matmul(lp[:, :w], ones_bf[:1, :],
                                     v1b_sb[:, lo:hi], start=False, stop=True)
                nc.scalar.activation(vocab_bf[:, lo:hi], lp[:, :w], AF.Exp,
                                     accum_out=psums[:, cix:cix + 1])
            zv = s4.tile([BL, 1], F32)
            nc.vector.tensor_reduce(zv[:], psums[:, :NCH],
                                    mybir.AxisListType.X, ALU.add)
            zv_inv = s4.tile([BL, 1], F32)
            nc.vector.reciprocal(zv_inv[:], zv[:])
            vscale = s4.tile([BL, 1], F32)
            nc.vector.tensor_tensor(vscale[:], zv_inv[:], pg_col[:], ALU.mult)
            nc.vector.tensor_scalar(vocab_bf[:], vocab_bf[:], vscale[:], None,
                                    ALU.mult)

        # ---------------- stage 4b: scatter + dense final ------------------
        with tc.tile_pool(name="sc", bufs=1) as scp, \
             tc.tile_pool(name="scs", bufs=2) as scs, \
             tc.tile_pool(name="cmbps", bufs=2, space="PSUM") as cmbps:
            zt = scp.tile([128, 1024], F32)
            nc.vector.memset(zt[:], 0.0)
            total = AROWS * 64
            zero_dmas = []
            lo = 0
            while lo < total:
                n = min(128 * 1024, total - lo)
                per = n // 128
                apz = bass.AP(adds_t, lo, [[per, 128], [1, per]])
                zero_dmas.append(nc.sync.dma_start(apz, zt[:, :per]))
                lo += n

            at_view = at_dram.ap().rearrange("b (q p) -> p (b q)", p=128, q=4)
            for gl in range(0, QCOLS, 8):
                nc.sync.dma_start(v_items[:, gl:gl + 8],
                                  at_view[:, gl:gl + 8])
            in_sc = scp.tile([128, QCOLS, 64], F32)
            for chk in range(QCOLS // 16):
                segc = scs.tile([128, 16, 128], BF, tag="segc")
                nc.sync.dma_start(segc[:],
                                  D["seg"][:, chk * 16:(chk + 1) * 16, :])
                ohc = scs.tile([128, 16, 64], BF, tag="ohc")
                nc.sync.dma_start(ohc[:], D["oh"][:, chk * 16:(chk + 1) * 16, :])
                for qq in range(16):
                    q = chk * 16 + qq
                    segv = scs.tile([128, 128], BF, tag="segv")
                    nc.vector.tensor_scalar(segv[:], segc[:, qq, :],
                                            v_items[:, q:q + 1], None, ALU.mult)
                    cmb = cmbps.tile([128, 64], F32, tag="cmb")
                    nc.tensor.matmul(cmb[:], segv[:], ohc[:, qq, :],
                                     start=True, stop=True)
                    nc.vector.tensor_copy(in_sc[:, q, :], cmb[:])

            idxs_sb = scp.tile([128, NIDX // 16], I16)
            nc.sync.dma_start(idxs_sb[:], D["idxs"][:])
            # SWDGE FIFO bounds one scatter at 2048 items -> 8 chunks of 16
            # columns. Each chunk's final slot is (b, q=3) m=127, which the
            # quota packing leaves unused (safe) -- the HW drops the last
            # descriptor of each scatter, so that slot must be sacrificial.
            sc_insts = []
            for c0 in range(0, QCOLS, 16):
                sc_inst = nc.gpsimd.dma_scatter_add(
                    out_ap=adds_t.ap()[:],
                    in_ap=in_sc[:, c0:c0 + 16, :],
                    idxs_ap=idxs_sb[:, c0 * 8:(c0 + 16) * 8],
                    num_idxs=2048, num_idxs_reg=2048,
                    elem_size=64, elem_step=64)
                for w in zero_dmas:
                    bass_rust.add_dep_helper(sc_inst.ins, w.ins, True,
                                             "scatter after zero")
                sc_insts.append(sc_inst)

            ez_sb = scp.tile([BL, OOV], F32)
            nc.sync.dma_start(ez_sb[:], D["ez"][:])
            adds_view = bass.AP(adds_t, 0, [[RPB * 64, BL], [1, VO]])
            CH = 1024
            for lo in range(0, VO, CH):
                hi = min(lo + CH, VO)
                w = hi - lo
                ach = scs.tile([BL, CH], F32, tag="ach")
                ld = nc.sync.dma_start(ach[:, :w], adds_view[:, lo:hi])
                for si in sc_insts:
                    bass_rust.add_dep_helper(ld.ins, si.ins, True,
                                             "dense after scatter")
                och = scs.tile([BL, CH], F32, tag="och")
                nc.vector.tensor_scalar(och[:, :w], ach[:, :w], ompg[:],
                                        None, ALU.mult)
                if lo < V:
                    vhi = min(hi, V)
                    nc.vector.tensor_tensor(och[:, :vhi - lo],
                                            och[:, :vhi - lo],
                                            vocab_bf[:, lo:vhi], ALU.add)
                if hi > V:
                    elo = max(lo, V)
                    nc.vector.tensor_tensor(
                        och[:, elo - lo:w], och[:, elo - lo:w],
                        ez_sb[:, elo - V:hi - V], ALU.add)
                nc.sync.dma_start(D["final"][:, lo:hi], och[:, :w])


# ------------------------------------------------------------------- run

_CACHE = {}


def kernel(**inputs):
    in_maps, v1bnz, perms = prep_inputs(inputs)
    key = ("v2", v1bnz)
    if key not in _CACHE:
        _CACHE[key] = build_program(v1bnz)
    nc = _CACHE[key]
    res = run_bass_kernel_spmd(nc, in_maps, list(range(NCORES)))
    return _assemble(inputs, res.results, perms)


def _assemble(inputs, results, perms):
    final = np.concatenate([r["final"] for r in results], 0)
    h_new = np.concatenate([r["h_new"] for r in results], 0)
    c_new = np.concatenate([r["c_new"] for r in results], 0)
    ct_e = np.concatenate([r["ct_e"] for r in results], 0)
    sum_new = np.empty((B, N), np.float32)
    for c in range(NCORES):
        s = slice(c * BL, (c + 1) * BL)
        np.put_along_axis(sum_new[s], perms[c], results[c]["sum_new"], axis=1)
    prev_s = np.asarray(inputs["prev_s"], np.float32)
    prev_s_new = np.concatenate([prev_s, h_new[:, None, :]], 1)
    return (final, h_new, c_new, ct_e, sum_new, prev_s_new)


# revision 20
# speedup vs baseline: 1.0129x; 1.0129x over previous
"""Trainium2 Bass kernel: pointer-generator decoder step (nn_Decoder).

Data-parallel over batch B=256 across 8 NeuronCores (32 rows/core). Host
pre-transposes/casts operands into matmul-friendly layouts and pre-computes
the scatter structure (segment/one-hot matrices) from the input index
tensor. Device computes: x-linear, LSTM cell, encoder intra-temporal
attention (the dominant [B*N,2H]x[2H,2H] matmul), decoder intra-attention,
pointer-generator head, 50k-vocab projection + softmax, and the scatter of
attention mass: per-column segment matmuls combine same-destination-row
contributions into 64-float rows, GPSIMD dma_scatter_add (overwrite
semantics on this stack) places them into a zero-initialized DRAM scratch
(one padded 783-row span per batch row so rows never alias across b), and a
dense final = base + (1-p_gen)*scratch pass produces the output.

The encoder n-axis is permuted per row on the host (applied consistently to
every n-indexed input; sum_new is un-permuted on output) so that scatter
destination groups are contiguous and bin-packable into whole columns.
"""

import numpy as np
import ml_dtypes

import concourse.bass as bass
import concourse.tile as tile
from concourse import bacc, mybir
from concourse.bass_utils import run_bass_kernel_spmd
from concourse.masks import make_identity
import bass_rust

B, N, H, E, V, OOV, T = 256, 400, 512, 256, 50000, 50, 49
NCORES = 8
BL = B // NCORES          # 32 rows per core
H2 = 2 * H                # 1024
VO = V + OOV              # 50050
QCOLS = 4 * BL            # 128 real scatter columns (4 per batch row)
NIDX = QCOLS * 128        # scatter slots (128 per column)
RPB = (VO + 63) // 64 + 1 # 783 scratch rows per batch row (no aliasing)
AROWS = 128 * ((BL * RPB + 1 + 127) // 128)  # padded scratch row count
SAFE = BL * RPB           # safe dummy row
TP = 64                   # padded T

BF = mybir.dt.bfloat16
F32 = mybir.dt.float32
I16 = mybir.dt.int16
bf16 = ml_dtypes.bfloat16

AF = mybir.ActivationFunctionType
ALU = mybir.AluOpType


# ---------------------------------------------------------------- host prep

def _to_tiles(mat, kt):
    nf = mat.shape[1]
    return np.ascontiguousarray(mat.reshape(kt, 128, nf).transpose(1, 0, 2))


def _pack_rhs(blocks, bias):
    mat = np.concatenate(blocks, 0)
    k, nf = mat.shape
    assert k % 128 == 0
    kt = k // 128 + 1
    out = np.zeros((kt * 128, nf), np.float32)
    out[:k] = mat
    out[k] = bias
    return _to_tiles(out, kt).astype(bf16)


def _packT(x, kt=None, ones_row=False):
    xt = x.T
    c = xt.shape[0]
    if kt is None:
        kt = (c + (1 if ones_row else 0) + 127) // 128
    out = np.zeros((kt * 128, xt.shape[1]), np.float32)
    out[:c] = xt
    if ones_row:
        out[c] = 1.0
    return _to_tiles(out, kt).astype(bf16)


def _scatter_struct(idx_rows):
    """idx_rows: [BL, N] raw destination columns. Computes the per-row
    permutation (sort by destination, then pack same-64-block groups into 4
    columns with EXACT quotas 128/128/128/16 so the compact layout is
    uniform across rows and cores) plus the device scatter structure.
    Returns (perm [BL, N], seg, oh, idxs)."""
    QUOTA = (128, 128, 128, 16)
    perm = np.empty((BL, N), np.int64)
    seg = np.zeros((128, QCOLS, 128), np.float32)
    oh = np.zeros((128, QCOLS, 64), np.float32)
    idxs_flat = np.full(NIDX, SAFE, np.int64)
    for b in range(BL):
        order = np.argsort(idx_rows[b], kind="stable")
        sidx = idx_rows[b][order]
        rows = sidx // 64
        bounds = [0] + list(np.nonzero(rows[1:] != rows[:-1])[0] + 1) + [N]
        groups = [(bounds[i], bounds[i + 1]) for i in range(len(bounds) - 1)]
        # first-fit decreasing into exact quotas (caps sum == N)
        cap = list(QUOTA)
        colgrp = [[], [], [], []]
        for gi in sorted(range(len(groups)),
                         key=lambda i: groups[i][1] - groups[i][0],
                         reverse=True):
            lo, hi = groups[gi]
            gl = hi - lo
            assert gl <= 128, "destination 64-block group too large"
            for q in range(4):
                if cap[q] >= gl:
                    cap[q] -= gl
                    colgrp[q].append(gi)
                    break
            else:
                raise AssertionError("quota packing failed")
        assert cap == [0, 0, 0, 0]
        for q in range(4):
            qg = 4 * b + q
            p = 0
            for m, gi in enumerate(sorted(colgrp[q])):
                lo, hi = groups[gi]
                idxs_flat[qg * 128 + m] = b * RPB + int(rows[lo])
                for src in range(lo, hi):
                    perm[b, q * 128 + p] = order[src]
                    seg[p, qg, m] = 1.0
                    oh[p, qg, sidx[src] % 64] = 1.0
                    p += 1
            assert p == QUOTA[q]
    idxs = np.zeros((16, NIDX // 16), np.int16)
    for i in range(NIDX):
        idxs[i % 16, i // 16] = idxs_flat[i]
    idxs = np.tile(idxs, (8, 1))          # replicated across the 8 Q7 cores
    return perm, seg.astype(bf16), oh.astype(bf16), idxs


def prep_inputs(inputs):
    g = {k: np.asarray(v) for k, v in inputs.items()}
    f32 = lambda k: np.asarray(g[k], np.float32)

    shared = {}
    shared["xiW"] = _pack_rhs([f32("x_input_w").T], f32("x_input_b"))
    shared["gatesW"] = _pack_rhs(
        [f32("lstm_w_ih").T, f32("lstm_w_hh").T],
        f32("lstm_b_ih") + f32("lstm_b_hh"))
    shared["wsT"] = _pack_rhs([f32("Ws_w").T], f32("Ws_b"))
    shared["wsdT"] = _pack_rhs([f32("Wsd_w").T], f32("Wsd_b"))
    shared["vwT"] = _pack_rhs([f32("V_w").T], f32("V_b"))
    shared["pgenW"] = _pack_rhs([f32("pgen_w").T], f32("pgen_b"))
    shared["whT"] = _to_tiles(f32("Wh_w").T, 8).astype(bf16)
    shared["wprevT"] = _to_tiles(f32("Wprev_w").T, 4).astype(bf16)
    v1t = np.zeros((512, 98 * 512), np.float32)
    v1t[:, :V] = f32("V1_w").T
    shared["v1T"] = np.ascontiguousarray(
        v1t.reshape(4, 128, 98, 512).transpose(1, 2, 0, 3)).astype(bf16)
    shared["vv"] = _to_tiles(f32("v_w").T, 8).astype(bf16)
    shared["vdv"] = _to_tiles(f32("vd_w").T, 4).astype(bf16)
    v1b = f32("V1_b")
    v1_bias_nonzero = bool(np.any(v1b))
    if v1_bias_nonzero:
        shared["v1b"] = np.ascontiguousarray(v1b[None, :]).astype(bf16)

    enc = f32("enc_out")
    prev = f32("prev_s")
    xct = np.concatenate([f32("x_t"), f32("ct_e")], axis=1)
    mask = f32("enc_padding_mask")
    sumt = f32("sum_temporal_srcs")
    minv = mask / sumt
    idx_all = np.asarray(g["enc_batch_extend_vocab"]).astype(np.int64)
    ez = f32("extra_zeros")

    in_maps = []
    perms = []
    for c in range(NCORES):
        s = slice(c * BL, (c + 1) * BL)
        m = dict(shared)
        perm, seg, oh, idxs = _scatter_struct(idx_all[s])
        perms.append(perm)
        encp = np.take_along_axis(enc[s], perm[:, :, None], axis=1)

        # [128, BL, 8, N]: per-b slice is one contiguous read per partition
        m["encT"] = np.ascontiguousarray(
            encp.transpose(2, 0, 1).reshape(8, 128, BL, N)
            .transpose(1, 2, 0, 3)).astype(bf16)
        encnat = np.zeros((BL, 512, H2), np.float32)
        encnat[:, :N] = encp
        # [BL, 128, 4, H2]: row p holds members p, 128+p, 256+p, 384+p
        m["encnat"] = np.ascontiguousarray(
            encnat.reshape(BL, 4, 128, H2).transpose(0, 2, 1, 3)).astype(bf16)
        m["prevT"] = np.ascontiguousarray(
            prev[s].transpose(2, 0, 1).reshape(4, 128, BL * T)
            .transpose(1, 0, 2)).astype(bf16)
        prevnat = np.zeros((BL, TP, H), np.float32)
        prevnat[:, :T] = prev[s]
        m["prevnat"] = prevnat.astype(bf16)
        m["xctT"] = _packT(xct[s], kt=11, ones_row=True)
        m["dechT"] = _packT(f32("dec_h")[s], kt=4)
        m["dec_c"] = np.ascontiguousarray(f32("dec_c")[s])
        m["minv"] = np.ascontiguousarray(
            np.take_along_axis(minv[s], perm, axis=1))
        m["sum_t"] = np.ascontiguousarray(
            np.take_along_axis(sumt[s], perm, axis=1))
        m["ez"] = np.ascontiguousarray(ez[s])
        m["seg"] = seg
        m["oh"] = oh
        m["idxs"] = idxs
        in_maps.append(m)

    return in_maps, v1_bias_nonzero, perms


# ------------------------------------------------------------ device build

def build_program(v1_bias_nonzero):
    nc = bacc.Bacc("TRN2", target_bir_lowering=False, debug=False)
    D = {}

    def din(name, shape, dt=BF):
        D[name] = nc.dram_tensor(name, list(shape), dt, kind="ExternalInput").ap()

    def dout(name, shape, dt=F32):
        D[name] = nc.dram_tensor(name, list(shape), dt, kind="ExternalOutput").ap()

    din("xiW", [128, 11, 256]); din("gatesW", [128, 7, 2048])
    din("wsT", [128, 9, 1024]); din("wsdT", [128, 5, 512])
    din("vwT", [128, 17, 512]); din("pgenW", [128, 23, 1])
    din("whT", [128, 8, 1024]); din("wprevT", [128, 4, 512])
    din("v1T", [128, 98, 4, 512]); din("vv", [128, 8, 1]); din("vdv", [128, 4, 1])
    if v1_bias_nonzero:
        din("v1b", [1, V])
    din("encT", [128, BL, 8, N]); din("encnat", [BL, 128, 4, H2])
    din("prevT", [128, 4, BL * T]); din("prevnat", [BL, TP, H])
    din("xctT", [128, 11, BL]); din("dechT", [128, 4, BL])
    din("dec_c", [BL, H], F32)
    din("minv", [BL, N], F32); din("sum_t", [BL, N], F32)
    din("ez", [BL, OOV], F32)
    din("seg", [128, QCOLS, 128]); din("oh", [128, QCOLS, 64])
    din("idxs", [128, NIDX // 16], I16)

    adds_t = nc.dram_tensor("adds", [AROWS, 64], F32)
    at_dram = nc.dram_tensor("at_dram", [BL, 512], F32)

    dout("final", [BL, VO]); dout("h_new", [BL, H]); dout("c_new", [BL, H])
    dout("ct_e", [BL, H2]); dout("sum_new", [BL, N])

    with tile.TileContext(nc) as tc:
        _body(nc, tc, D, adds_t, at_dram, v1_bias_nonzero)

    nc.compile()
    return nc


def _mm_affine(nc, out_ps, lhs_tiles, rhs_sb, nf_slice, ones_tile):
    kt = len(lhs_tiles) + 1
    for k in range(kt):
        lhs = lhs_tiles[k] if k < len(lhs_tiles) else ones_tile
        nc.tensor.matmul(out_ps, lhs, rhs_sb[:, k, nf_slice],
                         start=(k == 0), stop=(k == kt - 1))


def _body(nc, tc, D, adds_t, at_dram, v1_bias_nonzero):
    from contextlib import ExitStack

    with ExitStack() as ctx:
        const = ctx.enter_context(tc.tile_pool(name="const", bufs=1))
        persist = ctx.enter_context(tc.tile_pool(name="persist", bufs=1))

        ident_bf = const.tile([128, 128], BF)
        make_identity(nc, ident_bf[:])
        ident_f = const.tile([128, 128], F32)
        make_identity(nc, ident_f[:])
        ones_bf = const.tile([128, BL], BF)       # row0 = 1 (bias K-tile)
        nc.gpsimd.memset(ones_bf[:], 0.0)
        nc.gpsimd.memset(ones_bf[:1, :], 1.0)
        ones_col = const.tile([128, 1], BF)       # all-ones (reduce lhsT)
        nc.gpsimd.memset(ones_col[:], 1.0)

        xT = persist.tile([128, 2, BL], BF)
        dechT = persist.tile([128, 4, BL], BF)
        nc.sync.dma_start(dechT[:], D["dechT"][:])
        hT = persist.tile([128, 4, BL], BF)
        stT = persist.tile([128, 8, BL], BF)
        sprojT = persist.tile([128, 8, BL], F32)
        hdprojT = persist.tile([128, 4, BL], F32)
        ct_eT_bf = persist.tile([128, 8, BL], BF)
        ct_dT_bf = persist.tile([128, 4, BL], BF)
        vv_sb = const.tile([128, 8, 1], BF)
        nc.sync.dma_start(vv_sb[:], D["vv"][:])
        vdv_sb = const.tile([128, 4, 1], BF)
        nc.sync.dma_start(vdv_sb[:], D["vdv"][:])
        exp_all = persist.tile([BL, N], F32)
        ct_e_sb = persist.tile([BL, H2], F32)
        ct_d_sb = persist.tile([BL, H], F32)
        v_items = persist.tile([128, QCOLS], F32)
        pg_col = persist.tile([BL, 1], F32)
        ompg = persist.tile([BL, 1], F32)

        # ---------------- stage 1 ----------------------------------------
        with tc.tile_pool(name="s1w", bufs=1) as s1w, \
             tc.tile_pool(name="s1", bufs=1) as s1, \
             tc.tile_pool(name="s1ps", bufs=1, space="PSUM") as s1ps, \
             tc.tile_pool(name="s1tp", bufs=2, space="PSUM") as tp_ps:
            xiW = s1w.tile([128, 11, 256], BF)
            nc.sync.dma_start(xiW[:], D["xiW"][:])
            gatesW = s1w.tile([128, 7, 2048], BF)
            nc.sync.dma_start(gatesW[:], D["gatesW"][:])
            wsT = s1w.tile([128, 9, 1024], BF)
            nc.sync.dma_start(wsT[:], D["wsT"][:])
            wsdT = s1w.tile([128, 5, 512], BF)
            nc.sync.dma_start(wsdT[:], D["wsdT"][:])
            xctT = s1w.tile([128, 11, BL], BF)
            nc.sync.dma_start(xctT[:], D["xctT"][:])

            x_ps = s1ps.tile([BL, 256], F32, tag="big")
            for k in range(11):
                nc.tensor.matmul(x_ps[:], xctT[:, k, :], xiW[:, k, :],
                                 start=(k == 0), stop=(k == 10))
            x_bf = s1.tile([BL, 256], BF)
            nc.vector.tensor_copy(x_bf[:], x_ps[:])
            for k in range(2):
                ps = tp_ps.tile([128, BL], BF, tag="tp")
                nc.tensor.transpose(ps[:], x_bf[:, k * 128:(k + 1) * 128],
                                    ident_bf[:BL, :BL])
                nc.vector.tensor_copy(xT[:, k, :], ps[:])

            lhs = [xT[:, k, :] for k in range(2)] + \
                  [dechT[:, k, :] for k in range(4)]
            gates_ps = s1ps.tile([BL, 2048], F32, tag="big")
            for j in range(4):
                _mm_affine(nc, gates_ps[:, j * 512:(j + 1) * 512], lhs, gatesW,
                           slice(j * 512, (j + 1) * 512), ones_bf[:])
            sig_i = s1.tile([BL, H], F32)
            nc.scalar.activation(sig_i[:], gates_ps[:, 0:H], AF.Sigmoid)
            sig_f = s1.tile([BL, H], F32)
            nc.scalar.activation(sig_f[:], gates_ps[:, H:2 * H], AF.Sigmoid)
            tanh_g = s1.tile([BL, H], F32)
            nc.scalar.activation(tanh_g[:], gates_ps[:, 2 * H:3 * H], AF.Tanh)
            sig_o = s1.tile([BL, H], F32)
            nc.scalar.activation(sig_o[:], gates_ps[:, 3 * H:4 * H], AF.Sigmoid)
            dec_c = s1.tile([BL, H], F32)
            nc.sync.dma_start(dec_c[:], D["dec_c"][:])
            t1 = s1.tile([BL, H], F32)
            nc.vector.tensor_tensor(t1[:], sig_f[:], dec_c[:], ALU.mult)
            t2 = s1.tile([BL, H], F32)
            nc.vector.tensor_tensor(t2[:], sig_i[:], tanh_g[:], ALU.mult)
            c_new = s1.tile([BL, H], F32)
            nc.vector.tensor_tensor(c_new[:], t1[:], t2[:], ALU.add)
            tanh_c = s1.tile([BL, H], F32)
            nc.scalar.activation(tanh_c[:], c_new[:], AF.Tanh)
            h_new = s1.tile([BL, H], F32)
            nc.vector.tensor_tensor(h_new[:], sig_o[:], tanh_c[:], ALU.mult)
            nc.sync.dma_start(D["h_new"][:], h_new[:])
            nc.sync.dma_start(D["c_new"][:], c_new[:])

            st_bf = s1.tile([BL, H2], BF)
            nc.vector.tensor_copy(st_bf[:, :H], h_new[:])
            nc.vector.tensor_copy(st_bf[:, H:], c_new[:])
            for k in range(8):
                ps = tp_ps.tile([128, BL], BF, tag="tp")
                nc.tensor.transpose(ps[:], st_bf[:, k * 128:(k + 1) * 128],
                                    ident_bf[:BL, :BL])
                nc.vector.tensor_copy(stT[:, k, :], ps[:])
            for k in range(4):
                nc.vector.tensor_copy(hT[:, k, :], stT[:, k, :])

            st_lhs = [stT[:, k, :] for k in range(8)]
            for j in range(2):
                sp_ps = s1ps.tile([BL, 512], F32, tag="big")
                _mm_affine(nc, sp_ps[:], st_lhs, wsT,
                           slice(j * 512, (j + 1) * 512), ones_bf[:])
                sp_sb = s1.tile([BL, 512], F32, tag="spsb")
                nc.vector.tensor_copy(sp_sb[:], sp_ps[:])
                for k in range(4):
                    ps = tp_ps.tile([128, BL], F32, tag="tpf")
                    nc.tensor.transpose(ps[:], sp_sb[:, k * 128:(k + 1) * 128],
                                        ident_f[:BL, :BL])
                    nc.vector.tensor_copy(sprojT[:, j * 4 + k, :], ps[:])

            h_lhs = [hT[:, k, :] for k in range(4)]
            hd_ps = s1ps.tile([BL, 512], F32, tag="big")
            _mm_affine(nc, hd_ps[:], h_lhs, wsdT, slice(0, 512), ones_bf[:])
            hd_sb = s1.tile([BL, 512], F32, tag="spsb")
            nc.vector.tensor_copy(hd_sb[:], hd_ps[:])
            for k in range(4):
                ps = tp_ps.tile([128, BL], F32, tag="tpf")
                nc.tensor.transpose(ps[:], hd_sb[:, k * 128:(k + 1) * 128],
                                    ident_f[:BL, :BL])
                nc.vector.tensor_copy(hdprojT[:, k, :], ps[:])

        # ---------------- stage 2: encoder attention ----------------------
        with tc.tile_pool(name="s2w", bufs=1) as s2w, \
             tc.tile_pool(name="s2", bufs=2) as s2, \
             tc.tile_pool(name="s2row", bufs=2) as s2row, \
             tc.tile_pool(name="zps", bufs=2, space="PSUM") as zps, \
             tc.tile_pool(name="etps", bufs=1, space="PSUM") as etps, \
             tc.tile_pool(name="ctps", bufs=1, space="PSUM") as ctps, \
             tc.tile_pool(name="tp2", bufs=1, space="PSUM") as tp2:
            whT = s2w.tile([128, 8, 1024], BF)
            nc.sync.dma_start(whT[:], D["whT"][:])

            for b in range(BL):
                encb = s2.tile([128, 8, N], BF, tag="encb")
                nc.sync.dma_start(encb[:], D["encT"][:, b])
                tz = s2.tile([128, 8, N], BF, tag="tz")
                for half in range(4):
                    zt = zps.tile([128, 2, 512], F32, tag="z")
                    for j in range(2):
                        em = half * 2 + j
                        for k in range(8):
                            nc.tensor.matmul(
                                zt[:, j, :N], whT[:, k, em * 128:(em + 1) * 128],
                                encb[:, k, :], start=(k == 0), stop=(k == 7))
                        nc.scalar.activation(tz[:, em, :], zt[:, j, :N], AF.Tanh,
                                             bias=sprojT[:, em, b:b + 1])
                et_ps = etps.tile([1, N], F32, tag="et")
                for k in range(8):
                    nc.tensor.matmul(et_ps[:], vv_sb[:, k, :], tz[:, k, :],
                                     start=(k == 0), stop=(k == 7))
                exp_row = s2row.tile([1, N], F32, tag="exp")
                nc.scalar.activation(exp_row[:], et_ps[:], AF.Exp)
                nc.sync.dma_start(exp_all[b:b + 1, :], exp_row[:])
                minv_row = s2row.tile([1, N], F32, tag="minv")
                nc.sync.dma_start(minv_row[:], D["minv"][b:b + 1, :])
                w_row = s2row.tile([1, N], F32, tag="w")
                nc.vector.tensor_tensor(w_row[:], exp_row[:], minv_row[:],
                                        ALU.mult)
                zsum = s2row.tile([1, 1], F32, tag="zsum")
                nc.vector.tensor_reduce(zsum[:], w_row[:],
                                        mybir.AxisListType.X, ALU.add)
                zinv = s2row.tile([1, 1], F32, tag="zinv")
                nc.vector.reciprocal(zinv[:], zsum[:])
                at_row = s2row.tile([1, 512], F32, tag="at")
                nc.vector.memset(at_row[:], 0.0)
                nc.vector.tensor_scalar(at_row[:, :N], w_row[:], zinv[:],
                                        None, ALU.mult)
                # at values -> DRAM bounce (gathered into v_items later)
                nc.sync.dma_start(at_dram.ap()[b:b + 1, :], at_row[:])
                # at as per-partition columns for the ct_e weighted reduce
                at_colT = s2row.tile([128, 4], F32, tag="atc")
                for k in range(4):
                    ps = tp2.tile([128, 1], F32, tag="tpa")
                    nc.tensor.transpose(ps[:],
                                        at_row[:1, k * 128:(k + 1) * 128],
                                        ident_f[:1, :1])
                    nc.vector.tensor_copy(at_colT[:, k:k + 1], ps[:])
                encnb = s2.tile([128, 4, H2], BF, tag="encnb")
                nc.sync.dma_start(encnb[:], D["encnat"][b])
                ct_ps = ctps.tile([1, H2], F32, tag="ct")
                for k in range(4):
                    prod = s2.tile([128, H2], BF, tag="prod")
                    nc.vector.tensor_scalar(prod[:], encnb[:, k, :],
                                            at_colT[:, k:k + 1], None, ALU.mult)
                    for j in range(2):
                        nc.tensor.matmul(
                            ct_ps[:, j * 512:(j + 1) * 512], ones_col[:],
                            prod[:, j * 512:(j + 1) * 512],
                            start=(k == 0), stop=(k == 3))
                ct_row = s2row.tile([1, H2], F32, tag="ctr")
                nc.vector.tensor_copy(ct_row[:], ct_ps[:])
                nc.sync.dma_start(ct_e_sb[b:b + 1, :], ct_row[:])

        nc.sync.dma_start(D["ct_e"][:], ct_e_sb[:])

        with tc.tile_pool(name="sn", bufs=1) as sn:
            sum_t = sn.tile([BL, N], F32)
            nc.sync.dma_start(sum_t[:], D["sum_t"][:])
            sum_new = sn.tile([BL, N], F32)
            nc.vector.tensor_tensor(sum_new[:], sum_t[:], exp_all[:], ALU.add)
            nc.sync.dma_start(D["sum_new"][:], sum_new[:])

        with tc.tile_pool(name="cte", bufs=1) as cte, \
             tc.tile_pool(name="ctetp", bufs=2, space="PSUM") as ctetp:
            ce_bf = cte.tile([BL, H2], BF)
            nc.vector.tensor_copy(ce_bf[:], ct_e_sb[:])
            for k in range(8):
                ps = ctetp.tile([128, BL], BF, tag="tp")
                nc.tensor.transpose(ps[:], ce_bf[:, k * 128:(k + 1) * 128],
                                    ident_bf[:BL, :BL])
                nc.vector.tensor_copy(ct_eT_bf[:, k, :], ps[:])

        # ---------------- stage 3: decoder intra-attention ----------------
        with tc.tile_pool(name="s3w", bufs=1) as s3w, \
             tc.tile_pool(name="s3", bufs=2) as s3, \
             tc.tile_pool(name="s3row", bufs=2) as s3row, \
             tc.tile_pool(name="s3zd", bufs=1, space="PSUM") as s3zd, \
             tc.tile_pool(name="s3ps", bufs=1, space="PSUM") as s3ps, \
             tc.tile_pool(name="s3tp", bufs=1, space="PSUM") as s3tp:
            wprevT = s3w.tile([128, 4, 512], BF)
            nc.sync.dma_start(wprevT[:], D["wprevT"][:])
            prevT = s3w.tile([128, 4, BL * T], BF)
            nc.sync.dma_start(prevT[:], D["prevT"][:])


            for b in range(BL):
                pb = prevT[:, :, b * T:(b + 1) * T]
                zd = s3zd.tile([128, 4, 512], F32, tag="zd")
                tzd = s3.tile([128, 4, T], BF, tag="tzd")
                for gq in range(4):
                    for k in range(4):
                        nc.tensor.matmul(
                            zd[:, gq, :T], wprevT[:, k, gq * 128:(gq + 1) * 128],
                            pb[:, k, :], start=(k == 0), stop=(k == 3))
                    nc.scalar.activation(tzd[:, gq, :], zd[:, gq, :T], AF.Tanh,
                                         bias=hdprojT[:, gq, b:b + 1])
                etd_ps = s3ps.tile([1, T], F32, tag="etd")
                for k in range(4):
                    nc.tensor.matmul(etd_ps[:], vdv_sb[:, k, :], tzd[:, k, :],
                                     start=(k == 0), stop=(k == 3))
                expd = s3row.tile([1, T], F32, tag="expd")
                zd_sum = s3row.tile([1, 1], F32, tag="zdsum")
                nc.scalar.activation(expd[:], etd_ps[:], AF.Exp,
                                     accum_out=zd_sum[:])
                zd_inv = s3row.tile([1, 1], F32, tag="zdinv")
                nc.vector.reciprocal(zd_inv[:], zd_sum[:])
                atd_row = s3row.tile([1, T], F32, tag="atd")
                nc.vector.tensor_scalar(atd_row[:], expd[:], zd_inv[:],
                                        None, ALU.mult)
                atd_ps = s3tp.tile([TP, 1], F32, tag="tpd")
                nc.tensor.transpose(atd_ps[:T, :], atd_row[:1, :],
                                    ident_f[:1, :1])
                atd_col = s3row.tile([TP, 1], F32, tag="atdc")
                nc.vector.memset(atd_col[:], 0.0)
                nc.vector.tensor_copy(atd_col[:T, :], atd_ps[:T, :])
                pnb = s3.tile([TP, H], BF, tag="pnb")
                nc.sync.dma_start(pnb[:], D["prevnat"][b])
                prodd = s3.tile([TP, H], BF, tag="prodd")
                nc.vector.tensor_scalar(prodd[:], pnb[:], atd_col[:],
                                        None, ALU.mult)
                ctd_ps = s3ps.tile([1, H], F32, tag="ctd")
                nc.tensor.matmul(ctd_ps[:], ones_col[:TP, :], prodd[:],
                                 start=True, stop=True)
                ctd_row = s3row.tile([1, H], F32, tag="ctdr")
                nc.vector.tensor_copy(ctd_row[:], ctd_ps[:])
                nc.sync.dma_start(ct_d_sb[b:b + 1, :], ctd_row[:])

        with tc.tile_pool(name="ctd", bufs=1) as ctdp, \
             tc.tile_pool(name="ctdtp", bufs=2, space="PSUM") as ctdtp:
            cd_bf = ctdp.tile([BL, H], BF)
            nc.vector.tensor_copy(cd_bf[:], ct_d_sb[:])
            for k in range(4):
                ps = ctdtp.tile([128, BL], BF, tag="tp")
                nc.tensor.transpose(ps[:], cd_bf[:, k * 128:(k + 1) * 128],
                                    ident_bf[:BL, :BL])
                nc.vector.tensor_copy(ct_dT_bf[:, k, :], ps[:])

        # ---------------- stage 4a: p_gen, out-vec, vocab ------------------
        vocab_pool = ctx.enter_context(tc.tile_pool(name="vocab", bufs=1))
        vocab_bf = vocab_pool.tile([BL, V], BF)

        with tc.tile_pool(name="s4w", bufs=1) as s4w, \
             tc.tile_pool(name="s4", bufs=1) as s4, \
             tc.tile_pool(name="v1s", bufs=3) as v1s, \
             tc.tile_pool(name="s4ps", bufs=1, space="PSUM") as s4ps, \
             tc.tile_pool(name="lps", bufs=3, space="PSUM") as lps, \
             tc.tile_pool(name="s4tp", bufs=2, space="PSUM") as tp_ps:
            vwT = s4w.tile([128, 17, 512], BF)
            nc.sync.dma_start(vwT[:], D["vwT"][:])
            pgenW = s4w.tile([128, 23, 1], BF)
            nc.sync.dma_start(pgenW[:], D["pgenW"][:])

            pg_lhs = ([ct_eT_bf[:, k, :] for k in range(8)]
                      + [ct_dT_bf[:, k, :] for k in range(4)]
                      + [stT[:, k, :] for k in range(8)]
                      + [xT[:, k, :] for k in range(2)])
            pg_ps = s4ps.tile([BL, 1], F32, tag="pg")
            _mm_affine(nc, pg_ps[:], pg_lhs, pgenW, slice(0, 1), ones_bf[:])
            nc.scalar.activation(pg_col[:], pg_ps[:], AF.Sigmoid)
            nc.vector.tensor_scalar(ompg[:], pg_col[:], -1.0, 1.0, ALU.mult,
                                    ALU.add)

            ov_lhs = ([hT[:, k, :] for k in range(4)]
                      + [ct_eT_bf[:, k, :] for k in range(8)]
                      + [ct_dT_bf[:, k, :] for k in range(4)])
            ov_ps = s4ps.tile([BL, 512], F32, tag="ov")
            _mm_affine(nc, ov_ps[:], ov_lhs, vwT, slice(0, 512), ones_bf[:])
            ov_bf = s4.tile([BL, 512], BF)
            nc.vector.tensor_copy(ov_bf[:], ov_ps[:])
            ovT = s4.tile([128, 4, BL], BF)
            for k in range(4):
                ps = tp_ps.tile([128, BL], BF, tag="tp")
                nc.tensor.transpose(ps[:], ov_bf[:, k * 128:(k + 1) * 128],
                                    ident_bf[:BL, :BL])
                nc.vector.tensor_copy(ovT[:, k, :], ps[:])

            NCH = (V + 511) // 512
            psums = s4.tile([BL, 128], F32)
            if v1_bias_nonzero:
                v1b_sb = s4w.tile([1, V], BF)
                nc.sync.dma_start(v1b_sb[:], D["v1b"][:])
            for cix in range(NCH):
                lo = cix * 512
                hi = min(lo + 512, V)
                w = hi - lo
                v1c = v1s.tile([128, 4, 512], BF, tag="v1c")
                nc.sync.dma_start(v1c[:], D["v1T"][:, cix])
                lp = lps.tile([BL, 512], F32, tag="lg")
                nkt = 5 if v1_bias_nonzero else 4
                for k in range(4):
                    nc.tensor.matmul(lp[:, :w], ovT[:, k, :], v1c[:, k, :w],
                                     start=(k == 0), stop=(k == nkt - 1))
                if v1_bias_nonzero:
                    nc.tensor.matmul(lp[:, :w], ones_bf[:1, :],
                                     v1b_sb[:, lo:hi], start=False, stop=True)
                nc.scalar.activation(vocab_bf[:, lo:hi], lp[:, :w], AF.Exp,
                                     accum_out=psums[:, cix:cix + 1])
            zv = s4.tile([BL, 1], F32)
            nc.vector.tensor_reduce(zv[:], psums[:, :NCH],
                                    mybir.AxisListType.X, ALU.add)
            zv_inv = s4.tile([BL, 1], F32)
            nc.vector.reciprocal(zv_inv[:], zv[:])
            vscale = s4.tile([BL, 1], F32)
            nc.vector.tensor_tensor(vscale[:], zv_inv[:], pg_col[:], ALU.mult)
            nc.vector.tensor_scalar(vocab_bf[:], vocab_bf[:], vscale[:], None,
                                    ALU.mult)

        # ---------------- stage 4b: scatter + dense final ------------------
        with tc.tile_pool(name="sc", bufs=1) as scp, \
             tc.tile_pool(name="scs", bufs=2) as scs, \
             tc.tile_pool(name="cmbps", bufs=2, space="PSUM") as cmbps:
            zt = scp.tile([128, 1024], F32)
            nc.vector.memset(zt[:], 0.0)
            total = AROWS * 64
            zero_dmas = []
            lo = 0
            while lo < total:
                n = min(128 * 1024, total - lo)
                per = n // 128
                apz = bass.AP(adds_t, lo, [[per, 128], [1, per]])
                zero_dmas.append(nc.sync.dma_start(apz, zt[:, :per]))
                lo += n

            at_view = at_dram.ap().rearrange("b (q p) -> p (b q)", p=128, q=4)
            for gl in range(0, QCOLS, 8):
                nc.sync.dma_start(v_items[:, gl:gl + 8],
                                  at_view[:, gl:gl + 8])
            in_sc = scp.tile([128, QCOLS, 64], F32)
            for chk in range(QCOLS // 16):
                segc = scs.tile([128, 16, 128], BF, tag="segc")
                nc.sync.dma_start(segc[:],
                                  D["seg"][:, chk * 16:(chk + 1) * 16, :])
                ohc = scs.tile([128, 16, 64], BF, tag="ohc")
                nc.sync.dma_start(ohc[:], D["oh"][:, chk * 16:(chk + 1) * 16, :])
                for qq in range(16):
                    q = chk * 16 + qq
                    segv = scs.tile([128, 128], BF, tag="segv")
                    nc.vector.tensor_scalar(segv[:], segc[:, qq, :],
                                            v_items[:, q:q + 1], None, ALU.mult)
                    cmb = cmbps.tile([128, 64], F32, tag="cmb")
                    nc.tensor.matmul(cmb[:], segv[:], ohc[:, qq, :],
                                     start=True, stop=True)
                    nc.vector.tensor_copy(in_sc[:, q, :], cmb[:])

            idxs_sb = scp.tile([128, NIDX // 16], I16)
            nc.sync.dma_start(idxs_sb[:], D["idxs"][:])
            # SWDGE FIFO bounds one scatter at 2048 items -> 8 chunks of 16
            # columns. Each chunk's final slot is (b, q=3) m=127, which the
            # quota packing leaves unused (safe) -- the HW drops the last
            # descriptor of each scatter, so that slot must be sacrificial.
            sc_insts = []
            for c0 in range(0, QCOLS, 16):
                sc_inst = nc.gpsimd.dma_scatter_add(
                    out_ap=adds_t.ap()[:],
                    in_ap=in_sc[:, c0:c0 + 16, :],
                    idxs_ap=idxs_sb[:, c0 * 8:(c0 + 16) * 8],
                    num_idxs=2048, num_idxs_reg=2048,
                    elem_size=64, elem_step=64)
                for w in zero_dmas:
                    bass_rust.add_dep_helper(sc_inst.ins, w.ins, True,
                                             "scatter after zero")
                sc_insts.append(sc_inst)

            ez_sb = scp.tile([BL, OOV], F32)
            nc.sync.dma_start(ez_sb[:], D["ez"][:])
            adds_view = bass.AP(adds_t, 0, [[RPB * 64, BL], [1, VO]])
            CH = 1024
            for lo in range(0, VO, CH):
                hi = min(lo + CH, VO)
                w = hi - lo
                ach = scs.tile([BL, CH], F32, tag="ach")
                ld = nc.sync.dma_start(ach[:, :w], adds_view[:, lo:hi])
                for si in sc_insts:
                    bass_rust.add_dep_helper(ld.ins, si.ins, True,
                                             "dense after scatter")
                och = scs.tile([BL, CH], F32, tag="och")
                nc.vector.tensor_scalar(och[:, :w], ach[:, :w], ompg[:],
                                        None, ALU.mult)
                if lo < V:
                    vhi = min(hi, V)
                    nc.vector.tensor_tensor(och[:, :vhi - lo],
                                            och[:, :vhi - lo],
                                            vocab_bf[:, lo:vhi], ALU.add)
                if hi > V:
                    elo = max(lo, V)
                    nc.vector.tensor_tensor(
                        och[:, elo - lo:w], och[:, elo - lo:w],
                        ez_sb[:, elo - V:hi - V], ALU.add)
                nc.sync.dma_start(D["final"][:, lo:hi], och[:, :w])


# ------------------------------------------------------------------- run

_CACHE = {}


def kernel(**inputs):
    in_maps, v1bnz, perms = prep_inputs(inputs)
    key = ("v2", v1bnz)
    if key not in _CACHE:
        _CACHE[key] = build_program(v1bnz)
    nc = _CACHE[key]
    res = run_bass_kernel_spmd(nc, in_maps, list(range(NCORES)))
    return _assemble(inputs, res.results, perms)


def _assemble(inputs, results, perms):
    final = np.concatenate([r["final"] for r in results], 0)
    h_new = np.concatenate([r["h_new"] for r in results], 0)
    c_new = np.concatenate([r["c_new"] for r in results], 0)
    ct_e = np.concatenate([r["ct_e"] for r in results], 0)
    sum_new = np.empty((B, N), np.float32)
    for c in range(NCORES):
        s = slice(c * BL, (c + 1) * BL)
        np.put_along_axis(sum_new[s], perms[c], results[c]["sum_new"], axis=1)
    prev_s = np.asarray(inputs["prev_s"], np.float32)
    prev_s_new = np.concatenate([prev_s, h_new[:, None, :]], 1)
    return (final, h_new, c_new, ct_e, sum_new, prev_s_new)


# revision 21
# speedup vs baseline: 1.5611x; 1.5412x over previous
"""Trainium2 Bass kernel: pointer-generator decoder step (nn_Decoder).

Data-parallel over batch B=256 across 8 NeuronCores (32 rows/core). Host
pre-transposes/casts operands into matmul-friendly layouts and pre-computes
the scatter structure (segment/one-hot matrices) from the input index
tensor. Device computes: x-linear, LSTM cell, encoder intra-temporal
attention (the dominant [B*N,2H]x[2H,2H] matmul), decoder intra-attention,
pointer-generator head, 50k-vocab projection + softmax, and the scatter of
attention mass: per-column segment matmuls combine same-destination-row
contributions into 64-float rows, GPSIMD dma_scatter_add (overwrite
semantics on this stack) places them into a zero-initialized DRAM scratch
(one padded 783-row span per batch row so rows never alias across b), and a
dense final = base + (1-p_gen)*scratch pass produces the output.

The encoder n-axis is permuted per row on the host (applied consistently to
every n-indexed input; sum_new is un-permuted on output) so that scatter
destination groups are contiguous and bin-packable into whole columns.
"""

import numpy as np
import ml_dtypes

import concourse.bass as bass
import concourse.tile as tile
from concourse import bacc, mybir
from concourse.bass_utils import run_bass_kernel_spmd
from concourse.masks import make_identity
import bass_rust

B, N, H, E, V, OOV, T = 256, 400, 512, 256, 50000, 50, 49
NCORES = 8
BL = B // NCORES          # 32 rows per core
H2 = 2 * H                # 1024
VO = V + OOV              # 50050
QCOLS = 4 * BL            # 128 real scatter columns (4 per batch row)
NIDX = QCOLS * 128        # scatter slots (128 per column)
RPB = (VO + 63) // 64 + 1 # 783 scratch rows per batch row (no aliasing)
AROWS = 128 * ((BL * RPB + 1 + 127) // 128)  # padded scratch row count
SAFE = BL * RPB           # safe dummy row
TP = 64                   # padded T

BF = mybir.dt.bfloat16
F32 = mybir.dt.float32
I16 = mybir.dt.int16
bf16 = ml_dtypes.bfloat16

AF = mybir.ActivationFunctionType
ALU = mybir.AluOpType


# ---------------------------------------------------------------- host prep

def _to_tiles(mat, kt):
    nf = mat.shape[1]
    return np.ascontiguousarray(mat.reshape(kt, 128, nf).transpose(1, 0, 2))


def _pack_rhs(blocks, bias):
    mat = np.concatenate(blocks, 0)
    k, nf = mat.shape
    assert k % 128 == 0
    kt = k // 128 + 1
    out = np.zeros((kt * 128, nf), np.float32)
    out[:k] = mat
    out[k] = bias
    return _to_tiles(out, kt).astype(bf16)


def _packT(x, kt=None, ones_row=False):
    xt = x.T
    c = xt.shape[0]
    if kt is None:
        kt = (c + (1 if ones_row else 0) + 127) // 128
    out = np.zeros((kt * 128, xt.shape[1]), np.float32)
    out[:c] = xt
    if ones_row:
        out[c] = 1.0
    return _to_tiles(out, kt).astype(bf16)


def _scatter_struct(idx_rows):
    """idx_rows: [BL, N] raw destination columns. Computes the per-row
    permutation (sort by destination, then pack same-64-block groups into 4
    columns with EXACT quotas 128/128/128/16 so the compact layout is
    uniform across rows and cores) plus the device scatter structure.
    Returns (perm [BL, N], seg, oh, idxs)."""
    QUOTA = (128, 128, 128, 16)
    perm = np.empty((BL, N), np.int64)
    seg = np.zeros((128, QCOLS, 128), np.float32)
    oh = np.zeros((128, QCOLS, 64), np.float32)
    idxs_flat = np.full(NIDX, SAFE, np.int64)
    for b in range(BL):
        order = np.argsort(idx_rows[b], kind="stable")
        sidx = idx_rows[b][order]
        rows = sidx // 64
        bounds = [0] + list(np.nonzero(rows[1:] != rows[:-1])[0] + 1) + [N]
        groups = [(bounds[i], bounds[i + 1]) for i in range(len(bounds) - 1)]
        # first-fit decreasing into exact quotas (caps sum == N)
        cap = list(QUOTA)
        colgrp = [[], [], [], []]
        for gi in sorted(range(len(groups)),
                         key=lambda i: groups[i][1] - groups[i][0],
                         reverse=True):
            lo, hi = groups[gi]
            gl = hi - lo
            assert gl <= 128, "destination 64-block group too large"
            for q in range(4):
                if cap[q] >= gl:
                    cap[q] -= gl
                    colgrp[q].append(gi)
                    break
            else:
                raise AssertionError("quota packing failed")
        assert cap == [0, 0, 0, 0]
        for q in range(4):
            qg = 4 * b + q
            p = 0
            for m, gi in enumerate(sorted(colgrp[q])):
                lo, hi = groups[gi]
                idxs_flat[qg * 128 + m] = b * RPB + int(rows[lo])
                for src in range(lo, hi):
                    perm[b, q * 128 + p] = order[src]
                    seg[p, qg, m] = 1.0
                    oh[p, qg, sidx[src] % 64] = 1.0
                    p += 1
            assert p == QUOTA[q]
    idxs = np.zeros((16, NIDX // 16), np.int16)
    for i in range(NIDX):
        idxs[i % 16, i // 16] = idxs_flat[i]
    idxs = np.tile(idxs, (8, 1))          # replicated across the 8 Q7 cores
    return perm, seg.astype(bf16), oh.astype(bf16), idxs


def prep_inputs(inputs):
    g = {k: np.asarray(v) for k, v in inputs.items()}
    f32 = lambda k: np.asarray(g[k], np.float32)

    shared = {}
    shared["xiW"] = _pack_rhs([f32("x_input_w").T], f32("x_input_b"))
    shared["gatesW"] = _pack_rhs(
        [f32("lstm_w_ih").T, f32("lstm_w_hh").T],
        f32("lstm_b_ih") + f32("lstm_b_hh"))
    shared["wsT"] = _pack_rhs([f32("Ws_w").T], f32("Ws_b"))
    shared["wsdT"] = _pack_rhs([f32("Wsd_w").T], f32("Wsd_b"))
    shared["vwT"] = _pack_rhs([f32("V_w").T], f32("V_b"))
    shared["pgenW"] = _pack_rhs([f32("pgen_w").T], f32("pgen_b"))
    shared["whT"] = _to_tiles(f32("Wh_w").T, 8).astype(bf16)
    shared["wprevT"] = _to_tiles(f32("Wprev_w").T, 4).astype(bf16)
    v1t = np.zeros((512, 98 * 512), np.float32)
    v1t[:, :V] = f32("V1_w").T
    shared["v1T"] = np.ascontiguousarray(
        v1t.reshape(4, 128, 98, 512).transpose(1, 2, 0, 3)).astype(bf16)
    shared["vv"] = _to_tiles(f32("v_w").T, 8).astype(bf16)
    shared["vdv"] = _to_tiles(f32("vd_w").T, 4).astype(bf16)
    v1b = f32("V1_b")
    v1_bias_nonzero = bool(np.any(v1b))
    if v1_bias_nonzero:
        shared["v1b"] = np.ascontiguousarray(v1b[None, :]).astype(bf16)

    enc = f32("enc_out")
    prev = f32("prev_s")
    xct = np.concatenate([f32("x_t"), f32("ct_e")], axis=1)
    mask = f32("enc_padding_mask")
    sumt = f32("sum_temporal_srcs")
    minv = mask / sumt
    idx_all = np.asarray(g["enc_batch_extend_vocab"]).astype(np.int64)
    ez = f32("extra_zeros")

    in_maps = []
    perms = []
    for c in range(NCORES):
        s = slice(c * BL, (c + 1) * BL)
        m = dict(shared)
        perm, seg, oh, idxs = _scatter_struct(idx_all[s])
        perms.append(perm)
        encp = np.take_along_axis(enc[s], perm[:, :, None], axis=1)

        # [128, BL, 8, N]: per-b slice is one contiguous read per partition
        m["encT"] = np.ascontiguousarray(
            encp.transpose(2, 0, 1).reshape(8, 128, BL, N)
            .transpose(1, 2, 0, 3)).astype(bf16)
        encnat = np.zeros((BL, 512, H2), np.float32)
        encnat[:, :N] = encp
        # [BL, 128, 4, H2]: row p holds members p, 128+p, 256+p, 384+p
        m["encnat"] = np.ascontiguousarray(
            encnat.reshape(BL, 4, 128, H2).transpose(0, 2, 1, 3)).astype(bf16)
        m["prevT"] = np.ascontiguousarray(
            prev[s].transpose(2, 0, 1).reshape(4, 128, BL * T)
            .transpose(1, 0, 2)).astype(bf16)
        prevnat = np.zeros((BL, TP, H), np.float32)
        prevnat[:, :T] = prev[s]
        m["prevnat"] = prevnat.astype(bf16)
        m["xctT"] = _packT(xct[s], kt=11, ones_row=True)
        m["dechT"] = _packT(f32("dec_h")[s], kt=4)
        m["dec_c"] = np.ascontiguousarray(f32("dec_c")[s])
        m["minv"] = np.ascontiguousarray(
            np.take_along_axis(minv[s], perm, axis=1))
        m["sum_t"] = np.ascontiguousarray(
            np.take_along_axis(sumt[s], perm, axis=1))
        m["ez"] = np.ascontiguousarray(ez[s])
        m["seg"] = seg
        m["oh"] = oh
        m["idxs"] = idxs
        in_maps.append(m)

    return in_maps, v1_bias_nonzero, perms


# ------------------------------------------------------------ device build

def build_program(v1_bias_nonzero):
    nc = bacc.Bacc("TRN2", target_bir_lowering=False, debug=False)
    D = {}

    def din(name, shape, dt=BF):
        D[name] = nc.dram_tensor(name, list(shape), dt, kind="ExternalInput").ap()

    def dout(name, shape, dt=F32):
        D[name] = nc.dram_tensor(name, list(shape), dt, kind="ExternalOutput").ap()

    din("xiW", [128, 11, 256]); din("gatesW", [128, 7, 2048])
    din("wsT", [128, 9, 1024]); din("wsdT", [128, 5, 512])
    din("vwT", [128, 17, 512]); din("pgenW", [128, 23, 1])
    din("whT", [128, 8, 1024]); din("wprevT", [128, 4, 512])
    din("v1T", [128, 98, 4, 512]); din("vv", [128, 8, 1]); din("vdv", [128, 4, 1])
    if v1_bias_nonzero:
        din("v1b", [1, V])
    din("encT", [128, BL, 8, N]); din("encnat", [BL, 128, 4, H2])
    din("prevT", [128, 4, BL * T]); din("prevnat", [BL, TP, H])
    din("xctT", [128, 11, BL]); din("dechT", [128, 4, BL])
    din("dec_c", [BL, H], F32)
    din("minv", [BL, N], F32); din("sum_t", [BL, N], F32)
    din("ez", [BL, OOV], F32)
    din("seg", [128, QCOLS, 128]); din("oh", [128, QCOLS, 64])
    din("idxs", [128, NIDX // 16], I16)

    adds_t = nc.dram_tensor("adds", [AROWS, 64], F32)
    at_dram = nc.dram_tensor("at_dram", [BL, 512], F32)

    dout("final", [BL, VO]); dout("h_new", [BL, H]); dout("c_new", [BL, H])
    dout("ct_e", [BL, H2]); dout("sum_new", [BL, N])

    with tile.TileContext(nc) as tc:
        _body(nc, tc, D, adds_t, at_dram, v1_bias_nonzero)

    nc.compile()
    return nc


def _mm_affine(nc, out_ps, lhs_tiles, rhs_sb, nf_slice, ones_tile):
    kt = len(lhs_tiles) + 1
    for k in range(kt):
        lhs = lhs_tiles[k] if k < len(lhs_tiles) else ones_tile
        nc.tensor.matmul(out_ps, lhs, rhs_sb[:, k, nf_slice],
                         start=(k == 0), stop=(k == kt - 1))


def _body(nc, tc, D, adds_t, at_dram, v1_bias_nonzero):
    from contextlib import ExitStack

    with ExitStack() as ctx:
        const = ctx.enter_context(tc.tile_pool(name="const", bufs=1))
        persist = ctx.enter_context(tc.tile_pool(name="persist", bufs=1))

        ident_bf = const.tile([128, 128], BF)
        make_identity(nc, ident_bf[:])
        ident_f = const.tile([128, 128], F32)
        make_identity(nc, ident_f[:])
        ones_bf = const.tile([128, BL], BF)       # row0 = 1 (bias K-tile)
        nc.gpsimd.memset(ones_bf[:], 0.0)
        nc.gpsimd.memset(ones_bf[:1, :], 1.0)
        ones_col = const.tile([128, 1], BF)       # all-ones (reduce lhsT)
        nc.gpsimd.memset(ones_col[:], 1.0)

        xT = persist.tile([128, 2, BL], BF)
        dechT = persist.tile([128, 4, BL], BF)
        nc.sync.dma_start(dechT[:], D["dechT"][:])
        hT = persist.tile([128, 4, BL], BF)
        stT = persist.tile([128, 8, BL], BF)
        sprojT = persist.tile([128, 8, BL], F32)
        hdprojT = persist.tile([128, 4, BL], F32)
        ct_eT_bf = persist.tile([128, 8, BL], BF)
        ct_dT_bf = persist.tile([128, 4, BL], BF)
        vv_sb = const.tile([128, 8, 1], BF)
        nc.sync.dma_start(vv_sb[:], D["vv"][:])
        vdv_sb = const.tile([128, 4, 1], BF)
        nc.sync.dma_start(vdv_sb[:], D["vdv"][:])
        exp_all = persist.tile([BL, N], F32)
        ct_e_sb = persist.tile([BL, H2], F32)
        ct_d_sb = persist.tile([BL, H], F32)
        v_items = persist.tile([128, QCOLS], F32)
        in_sc = persist.tile([128, QCOLS, 64], F32)
        idxs_sb = persist.tile([128, NIDX // 16], I16)
        nc.sync.dma_start(idxs_sb[:], D["idxs"][:])
        pg_col = persist.tile([BL, 1], F32)
        ompg = persist.tile([BL, 1], F32)
        # zero the scatter scratch up front
        ztile = persist.tile([128, 1024], F32)
        nc.vector.memset(ztile[:], 0.0)
        zero_dmas = []
        _lo = 0
        _total = AROWS * 64
        while _lo < _total:
            _n = min(128 * 1024, _total - _lo)
            _per = _n // 128
            apz = bass.AP(adds_t, _lo, [[_per, 128], [1, _per]])
            zero_dmas.append(nc.sync.dma_start(apz, ztile[:, :_per]))
            _lo += _n
        sc_insts = []

        # ---------------- stage 1 ----------------------------------------
        with tc.tile_pool(name="s1w", bufs=1) as s1w, \
             tc.tile_pool(name="s1", bufs=1) as s1, \
             tc.tile_pool(name="s1ps", bufs=1, space="PSUM") as s1ps, \
             tc.tile_pool(name="s1tp", bufs=2, space="PSUM") as tp_ps:
            xiW = s1w.tile([128, 11, 256], BF)
            nc.sync.dma_start(xiW[:], D["xiW"][:])
            gatesW = s1w.tile([128, 7, 2048], BF)
            nc.sync.dma_start(gatesW[:], D["gatesW"][:])
            wsT = s1w.tile([128, 9, 1024], BF)
            nc.sync.dma_start(wsT[:], D["wsT"][:])
            wsdT = s1w.tile([128, 5, 512], BF)
            nc.sync.dma_start(wsdT[:], D["wsdT"][:])
            xctT = s1w.tile([128, 11, BL], BF)
            nc.sync.dma_start(xctT[:], D["xctT"][:])

            x_ps = s1ps.tile([BL, 256], F32, tag="big")
            for k in range(11):
                nc.tensor.matmul(x_ps[:], xctT[:, k, :], xiW[:, k, :],
                                 start=(k == 0), stop=(k == 10))
            x_bf = s1.tile([BL, 256], BF)
            nc.vector.tensor_copy(x_bf[:], x_ps[:])
            for k in range(2):
                ps = tp_ps.tile([128, BL], BF, tag="tp")
                nc.tensor.transpose(ps[:], x_bf[:, k * 128:(k + 1) * 128],
                                    ident_bf[:BL, :BL])
                nc.vector.tensor_copy(xT[:, k, :], ps[:])

            lhs = [xT[:, k, :] for k in range(2)] + \
                  [dechT[:, k, :] for k in range(4)]
            gates_ps = s1ps.tile([BL, 2048], F32, tag="big")
            for j in range(4):
                _mm_affine(nc, gates_ps[:, j * 512:(j + 1) * 512], lhs, gatesW,
                           slice(j * 512, (j + 1) * 512), ones_bf[:])
            sig_i = s1.tile([BL, H], F32)
            nc.scalar.activation(sig_i[:], gates_ps[:, 0:H], AF.Sigmoid)
            sig_f = s1.tile([BL, H], F32)
            nc.scalar.activation(sig_f[:], gates_ps[:, H:2 * H], AF.Sigmoid)
            tanh_g = s1.tile([BL, H], F32)
            nc.scalar.activation(tanh_g[:], gates_ps[:, 2 * H:3 * H], AF.Tanh)
            sig_o = s1.tile([BL, H], F32)
            nc.scalar.activation(sig_o[:], gates_ps[:, 3 * H:4 * H], AF.Sigmoid)
            dec_c = s1.tile([BL, H], F32)
            nc.sync.dma_start(dec_c[:], D["dec_c"][:])
            t1 = s1.tile([BL, H], F32)
            nc.vector.tensor_tensor(t1[:], sig_f[:], dec_c[:], ALU.mult)
            t2 = s1.tile([BL, H], F32)
            nc.vector.tensor_tensor(t2[:], sig_i[:], tanh_g[:], ALU.mult)
            c_new = s1.tile([BL, H], F32)
            nc.vector.tensor_tensor(c_new[:], t1[:], t2[:], ALU.add)
            tanh_c = s1.tile([BL, H], F32)
            nc.scalar.activation(tanh_c[:], c_new[:], AF.Tanh)
            h_new = s1.tile([BL, H], F32)
            nc.vector.tensor_tensor(h_new[:], sig_o[:], tanh_c[:], ALU.mult)
            nc.sync.dma_start(D["h_new"][:], h_new[:])
            nc.sync.dma_start(D["c_new"][:], c_new[:])

            st_bf = s1.tile([BL, H2], BF)
            nc.vector.tensor_copy(st_bf[:, :H], h_new[:])
            nc.vector.tensor_copy(st_bf[:, H:], c_new[:])
            for k in range(8):
                ps = tp_ps.tile([128, BL], BF, tag="tp")
                nc.tensor.transpose(ps[:], st_bf[:, k * 128:(k + 1) * 128],
                                    ident_bf[:BL, :BL])
                nc.vector.tensor_copy(stT[:, k, :], ps[:])
            for k in range(4):
                nc.vector.tensor_copy(hT[:, k, :], stT[:, k, :])

            st_lhs = [stT[:, k, :] for k in range(8)]
            for j in range(2):
                sp_ps = s1ps.tile([BL, 512], F32, tag="big")
                _mm_affine(nc, sp_ps[:], st_lhs, wsT,
                           slice(j * 512, (j + 1) * 512), ones_bf[:])
                sp_sb = s1.tile([BL, 512], F32, tag="spsb")
                nc.vector.tensor_copy(sp_sb[:], sp_ps[:])
                for k in range(4):
                    ps = tp_ps.tile([128, BL], F32, tag="tpf")
                    nc.tensor.transpose(ps[:], sp_sb[:, k * 128:(k + 1) * 128],
                                        ident_f[:BL, :BL])
                    nc.vector.tensor_copy(sprojT[:, j * 4 + k, :], ps[:])

            h_lhs = [hT[:, k, :] for k in range(4)]
            hd_ps = s1ps.tile([BL, 512], F32, tag="big")
            _mm_affine(nc, hd_ps[:], h_lhs, wsdT, slice(0, 512), ones_bf[:])
            hd_sb = s1.tile([BL, 512], F32, tag="spsb")
            nc.vector.tensor_copy(hd_sb[:], hd_ps[:])
            for k in range(4):
                ps = tp_ps.tile([128, BL], F32, tag="tpf")
                nc.tensor.transpose(ps[:], hd_sb[:, k * 128:(k + 1) * 128],
                                    ident_f[:BL, :BL])
                nc.vector.tensor_copy(hdprojT[:, k, :], ps[:])

        # ---------------- stage 2: encoder attention ----------------------
        with tc.tile_pool(name="s2w", bufs=1) as s2w, \
             tc.tile_pool(name="s2", bufs=2) as s2, \
             tc.tile_pool(name="s2row", bufs=2) as s2row, \
             tc.tile_pool(name="scse", bufs=2) as scse, \
             tc.tile_pool(name="zps", bufs=2, space="PSUM") as zps, \
             tc.tile_pool(name="etps", bufs=1, space="PSUM") as etps, \
             tc.tile_pool(name="ctps", bufs=1, space="PSUM") as ctps, \
             tc.tile_pool(name="cmbps", bufs=2, space="PSUM") as cmbps, \
             tc.tile_pool(name="tp2", bufs=1, space="PSUM") as tp2:
            whT = s2w.tile([128, 8, 1024], BF)
            nc.sync.dma_start(whT[:], D["whT"][:])

            for b in range(BL):
                encb = s2.tile([128, 8, N], BF, tag="encb")
                nc.sync.dma_start(encb[:], D["encT"][:, b])
                tz = s2.tile([128, 8, N], BF, tag="tz")
                for em in range(8):
                    zt = zps.tile([128, 512], F32, tag="z")
                    for k in range(8):
                        nc.tensor.matmul(
                            zt[:, :N], whT[:, k, em * 128:(em + 1) * 128],
                            encb[:, k, :], start=(k == 0), stop=(k == 7))
                    nc.scalar.activation(tz[:, em, :], zt[:, :N], AF.Tanh,
                                         bias=sprojT[:, em, b:b + 1])
                et_ps = etps.tile([1, N], F32, tag="et")
                for k in range(8):
                    nc.tensor.matmul(et_ps[:], vv_sb[:, k, :], tz[:, k, :],
                                     start=(k == 0), stop=(k == 7))
                exp_row = s2row.tile([1, N], F32, tag="exp")
                nc.scalar.activation(exp_row[:], et_ps[:], AF.Exp)
                nc.sync.dma_start(exp_all[b:b + 1, :], exp_row[:])
                minv_row = s2row.tile([1, N], F32, tag="minv")
                nc.sync.dma_start(minv_row[:], D["minv"][b:b + 1, :])
                w_row = s2row.tile([1, N], F32, tag="w")
                nc.vector.tensor_tensor(w_row[:], exp_row[:], minv_row[:],
                                        ALU.mult)
                zsum = s2row.tile([1, 1], F32, tag="zsum")
                nc.vector.tensor_reduce(zsum[:], w_row[:],
                                        mybir.AxisListType.X, ALU.add)
                zinv = s2row.tile([1, 1], F32, tag="zinv")
                nc.vector.reciprocal(zinv[:], zsum[:])
                at_row = s2row.tile([1, 512], F32, tag="at")
                nc.vector.memset(at_row[:], 0.0)
                nc.vector.tensor_scalar(at_row[:, :N], w_row[:], zinv[:],
                                        None, ALU.mult)
                # at values -> DRAM bounce -> scatter item columns
                atw = nc.sync.dma_start(at_dram.ap()[b:b + 1, :], at_row[:])
                atr = nc.sync.dma_start(
                    v_items[:, 4 * b:4 * b + 4],
                    at_dram.ap()[b].rearrange("(q p) -> p q", p=128, q=4))
                bass_rust.add_dep_helper(atr.ins, atw.ins, True,
                                         "at bounce RAW")
                segb = scse.tile([128, 4, 128], BF, tag="segb")
                nc.sync.dma_start(segb[:], D["seg"][:, 4 * b:4 * b + 4, :])
                ohb = scse.tile([128, 4, 64], BF, tag="ohb")
                nc.sync.dma_start(ohb[:], D["oh"][:, 4 * b:4 * b + 4, :])
                for j in range(4):
                    q = 4 * b + j
                    segv = scse.tile([128, 128], BF, tag="segv")
                    nc.vector.tensor_scalar(segv[:], segb[:, j, :],
                                            v_items[:, q:q + 1], None, ALU.mult)
                    cmb = cmbps.tile([128, 64], F32, tag="cmb")
                    nc.tensor.matmul(cmb[:], segv[:], ohb[:, j, :],
                                     start=True, stop=True)
                    nc.vector.tensor_copy(in_sc[:, q, :], cmb[:])
                sci = nc.gpsimd.dma_scatter_add(
                    out_ap=adds_t.ap()[:],
                    in_ap=in_sc[:, 4 * b:4 * b + 4, :],
                    idxs_ap=idxs_sb[:, 32 * b:32 * b + 32],
                    num_idxs=512, num_idxs_reg=512,
                    elem_size=64, elem_step=64)
                for w in zero_dmas:
                    bass_rust.add_dep_helper(sci.ins, w.ins, True,
                                             "scatter after zero")
                sc_insts.append(sci)
                # at as per-partition columns for the ct_e weighted reduce
                at_colT = s2row.tile([128, 4], F32, tag="atc")
                for k in range(4):
                    ps = tp2.tile([128, 1], F32, tag="tpa")
                    nc.tensor.transpose(ps[:],
                                        at_row[:1, k * 128:(k + 1) * 128],
                                        ident_f[:1, :1])
                    nc.vector.tensor_copy(at_colT[:, k:k + 1], ps[:])
                encnb = s2.tile([128, 4, H2], BF, tag="encnb")
                nc.sync.dma_start(encnb[:], D["encnat"][b])
                ct_ps = ctps.tile([1, H2], F32, tag="ct")
                for k in range(4):
                    prod = s2.tile([128, H2], BF, tag="prod")
                    nc.vector.tensor_scalar(prod[:], encnb[:, k, :],
                                            at_colT[:, k:k + 1], None, ALU.mult)
                    for j in range(2):
                        nc.tensor.matmul(
                            ct_ps[:, j * 512:(j + 1) * 512], ones_col[:],
                            prod[:, j * 512:(j + 1) * 512],
                            start=(k == 0), stop=(k == 3))
                ct_row = s2row.tile([1, H2], F32, tag="ctr")
                nc.vector.tensor_copy(ct_row[:], ct_ps[:])
                nc.sync.dma_start(ct_e_sb[b:b + 1, :], ct_row[:])

        nc.sync.dma_start(D["ct_e"][:], ct_e_sb[:])

        with tc.tile_pool(name="sn", bufs=1) as sn:
            sum_t = sn.tile([BL, N], F32)
            nc.sync.dma_start(sum_t[:], D["sum_t"][:])
            sum_new = sn.tile([BL, N], F32)
            nc.vector.tensor_tensor(sum_new[:], sum_t[:], exp_all[:], ALU.add)
            nc.sync.dma_start(D["sum_new"][:], sum_new[:])

        with tc.tile_pool(name="cte", bufs=1) as cte, \
             tc.tile_pool(name="ctetp", bufs=2, space="PSUM") as ctetp:
            ce_bf = cte.tile([BL, H2], BF)
            nc.vector.tensor_copy(ce_bf[:], ct_e_sb[:])
            for k in range(8):
                ps = ctetp.tile([128, BL], BF, tag="tp")
                nc.tensor.transpose(ps[:], ce_bf[:, k * 128:(k + 1) * 128],
                                    ident_bf[:BL, :BL])
                nc.vector.tensor_copy(ct_eT_bf[:, k, :], ps[:])

        # ---------------- stage 3: decoder intra-attention ----------------
        with tc.tile_pool(name="s3w", bufs=1) as s3w, \
             tc.tile_pool(name="s3", bufs=2) as s3, \
             tc.tile_pool(name="s3row", bufs=2) as s3row, \
             tc.tile_pool(name="s3zd", bufs=1, space="PSUM") as s3zd, \
             tc.tile_pool(name="s3ps", bufs=1, space="PSUM") as s3ps, \
             tc.tile_pool(name="s3tp", bufs=1, space="PSUM") as s3tp:
            wprevT = s3w.tile([128, 4, 512], BF)
            nc.sync.dma_start(wprevT[:], D["wprevT"][:])
            prevT = s3w.tile([128, 4, BL * T], BF)
            nc.sync.dma_start(prevT[:], D["prevT"][:])


            for b in range(BL):
                pb = prevT[:, :, b * T:(b + 1) * T]
                zd = s3zd.tile([128, 4, 512], F32, tag="zd")
                tzd = s3.tile([128, 4, T], BF, tag="tzd")
                for gq in range(4):
                    for k in range(4):
                        nc.tensor.matmul(
                            zd[:, gq, :T], wprevT[:, k, gq * 128:(gq + 1) * 128],
                            pb[:, k, :], start=(k == 0), stop=(k == 3))
                    nc.scalar.activation(tzd[:, gq, :], zd[:, gq, :T], AF.Tanh,
                                         bias=hdprojT[:, gq, b:b + 1])
                etd_ps = s3ps.tile([1, T], F32, tag="etd")
                for k in range(4):
                    nc.tensor.matmul(etd_ps[:], vdv_sb[:, k, :], tzd[:, k, :],
                                     start=(k == 0), stop=(k == 3))
                expd = s3row.tile([1, T], F32, tag="expd")
                zd_sum = s3row.tile([1, 1], F32, tag="zdsum")
                nc.scalar.activation(expd[:], etd_ps[:], AF.Exp,
                                     accum_out=zd_sum[:])
                zd_inv = s3row.tile([1, 1], F32, tag="zdinv")
                nc.vector.reciprocal(zd_inv[:], zd_sum[:])
                atd_row = s3row.tile([1, T], F32, tag="atd")
                nc.vector.tensor_scalar(atd_row[:], expd[:], zd_inv[:],
                                        None, ALU.mult)
                atd_ps = s3tp.tile([TP, 1], F32, tag="tpd")
                nc.tensor.transpose(atd_ps[:T, :], atd_row[:1, :],
                                    ident_f[:1, :1])
                atd_col = s3row.tile([TP, 1], F32, tag="atdc")
                nc.vector.memset(atd_col[:], 0.0)
                nc.vector.tensor_copy(atd_col[:T, :], atd_ps[:T, :])
                pnb = s3.tile([TP, H], BF, tag="pnb")
                nc.sync.dma_start(pnb[:], D["prevnat"][b])
                prodd = s3.tile([TP, H], BF, tag="prodd")
                nc.vector.tensor_scalar(prodd[:], pnb[:], atd_col[:],
                                        None, ALU.mult)
                ctd_ps = s3ps.tile([1, H], F32, tag="ctd")
                nc.tensor.matmul(ctd_ps[:], ones_col[:TP, :], prodd[:],
                                 start=True, stop=True)
                ctd_row = s3row.tile([1, H], F32, tag="ctdr")
                nc.vector.tensor_copy(ctd_row[:], ctd_ps[:])
                nc.sync.dma_start(ct_d_sb[b:b + 1, :], ctd_row[:])

        with tc.tile_pool(name="ctd", bufs=1) as ctdp, \
             tc.tile_pool(name="ctdtp", bufs=2, space="PSUM") as ctdtp:
            cd_bf = ctdp.tile([BL, H], BF)
            nc.vector.tensor_copy(cd_bf[:], ct_d_sb[:])
            for k in range(4):
                ps = ctdtp.tile([128, BL], BF, tag="tp")
                nc.tensor.transpose(ps[:], cd_bf[:, k * 128:(k + 1) * 128],
                                    ident_bf[:BL, :BL])
                nc.vector.tensor_copy(ct_dT_bf[:, k, :], ps[:])

        # ---------------- stage 4a: p_gen, out-vec, vocab ------------------
        vocab_pool = ctx.enter_context(tc.tile_pool(name="vocab", bufs=1))
        vocab_bf = vocab_pool.tile([BL, V], BF)

        with tc.tile_pool(name="s4w", bufs=1) as s4w, \
             tc.tile_pool(name="s4", bufs=1) as s4, \
             tc.tile_pool(name="v1s", bufs=3) as v1s, \
             tc.tile_pool(name="s4ps", bufs=1, space="PSUM") as s4ps, \
             tc.tile_pool(name="lps", bufs=3, space="PSUM") as lps, \
             tc.tile_pool(name="s4tp", bufs=2, space="PSUM") as tp_ps:
            vwT = s4w.tile([128, 17, 512], BF)
            nc.sync.dma_start(vwT[:], D["vwT"][:])
            pgenW = s4w.tile([128, 23, 1], BF)
            nc.sync.dma_start(pgenW[:], D["pgenW"][:])

            pg_lhs = ([ct_eT_bf[:, k, :] for k in range(8)]
                      + [ct_dT_bf[:, k, :] for k in range(4)]
                      + [stT[:, k, :] for k in range(8)]
                      + [xT[:, k, :] for k in range(2)])
            pg_ps = s4ps.tile([BL, 1], F32, tag="pg")
            _mm_affine(nc, pg_ps[:], pg_lhs, pgenW, slice(0, 1), ones_bf[:])
            nc.scalar.activation(pg_col[:], pg_ps[:], AF.Sigmoid)
            nc.vector.tensor_scalar(ompg[:], pg_col[:], -1.0, 1.0, ALU.mult,
                                    ALU.add)

            ov_lhs = ([hT[:, k, :] for k in range(4)]
                      + [ct_eT_bf[:, k, :] for k in range(8)]
                      + [ct_dT_bf[:, k, :] for k in range(4)])
            ov_ps = s4ps.tile([BL, 512], F32, tag="ov")
            _mm_affine(nc, ov_ps[:], ov_lhs, vwT, slice(0, 512), ones_bf[:])
            ov_bf = s4.tile([BL, 512], BF)
            nc.vector.tensor_copy(ov_bf[:], ov_ps[:])
            ovT = s4.tile([128, 4, BL], BF)
            for k in range(4):
                ps = tp_ps.tile([128, BL], BF, tag="tp")
                nc.tensor.transpose(ps[:], ov_bf[:, k * 128:(k + 1) * 128],
                                    ident_bf[:BL, :BL])
                nc.vector.tensor_copy(ovT[:, k, :], ps[:])

            NCH = (V + 511) // 512
            psums = s4.tile([BL, 128], F32)
            if v1_bias_nonzero:
                v1b_sb = s4w.tile([1, V], BF)
                nc.sync.dma_start(v1b_sb[:], D["v1b"][:])
            for cix in range(NCH):
                lo = cix * 512
                hi = min(lo + 512, V)
                w = hi - lo
                v1c = v1s.tile([128, 4, 512], BF, tag="v1c")
                nc.sync.dma_start(v1c[:], D["v1T"][:, cix])
                lp = lps.tile([BL, 512], F32, tag="lg")
                nkt = 5 if v1_bias_nonzero else 4
                for k in range(4):
                    nc.tensor.matmul(lp[:, :w], ovT[:, k, :], v1c[:, k, :w],
                                     start=(k == 0), stop=(k == nkt - 1))
                if v1_bias_nonzero:
                    nc.tensor.matmul(lp[:, :w], ones_bf[:1, :],
                                     v1b_sb[:, lo:hi], start=False, stop=True)
                nc.scalar.activation(vocab_bf[:, lo:hi], lp[:, :w], AF.Exp,
                                     accum_out=psums[:, cix:cix + 1])
            zv = s4.tile([BL, 1], F32)
            nc.vector.tensor_reduce(zv[:], psums[:, :NCH],
                                    mybir.AxisListType.X, ALU.add)
            zv_inv = s4.tile([BL, 1], F32)
            nc.vector.reciprocal(zv_inv[:], zv[:])
            vscale = s4.tile([BL, 1], F32)
            nc.vector.tensor_tensor(vscale[:], zv_inv[:], pg_col[:], ALU.mult)
            nc.vector.tensor_scalar(vocab_bf[:], vocab_bf[:], vscale[:], None,
                                    ALU.mult)

        # ---------------- stage 4b: dense final ----------------------------
        with tc.tile_pool(name="sc", bufs=1) as scp, \
             tc.tile_pool(name="scs", bufs=2) as scs:
            ez_sb = scp.tile([BL, OOV], F32)
            nc.sync.dma_start(ez_sb[:], D["ez"][:])
            adds_view = bass.AP(adds_t, 0, [[RPB * 64, BL], [1, VO]])
            CH = 1024
            for lo in range(0, VO, CH):
                hi = min(lo + CH, VO)
                w = hi - lo
                ach = scs.tile([BL, CH], F32, tag="ach")
                ld = nc.sync.dma_start(ach[:, :w], adds_view[:, lo:hi])
                for si in sc_insts:
                    bass_rust.add_dep_helper(ld.ins, si.ins, True,
                                             "dense after scatter")
                och = scs.tile([BL, CH], F32, tag="och")
                nc.vector.tensor_scalar(och[:, :w], ach[:, :w], ompg[:],
                                        None, ALU.mult)
                if lo < V:
                    vhi = min(hi, V)
                    nc.vector.tensor_tensor(och[:, :vhi - lo],
                                            och[:, :vhi - lo],
                                            vocab_bf[:, lo:vhi], ALU.add)
                if hi > V:
                    elo = max(lo, V)
                    nc.vector.tensor_tensor(
                        och[:, elo - lo:w], och[:, elo - lo:w],
                        ez_sb[:, elo - V:hi - V], ALU.add)
                nc.sync.dma_start(D["final"][:, lo:hi], och[:, :w])


# ------------------------------------------------------------------- run

_CACHE = {}


def kernel(**inputs):
    in_maps, v1bnz, perms = prep_inputs(inputs)
    key = ("v2", v1bnz)
    if key not in _CACHE:
        _CACHE[key] = build_program(v1bnz)
    nc = _CACHE[key]
    res = run_bass_kernel_spmd(nc, in_maps, list(range(NCORES)))
    return _assemble(inputs, res.results, perms)


def _assemble(inputs, results, perms):
    final = np.concatenate([r["final"] for r in results], 0)
    h_new = np.concatenate([r["h_new"] for r in results], 0)
    c_new = np.concatenate([r["c_new"] for r in results], 0)
    ct_e = np.concatenate([r["ct_e"] for r in results], 0)
    sum_new = np.empty((B, N), np.float32)
    for c in range(NCORES):
        s = slice(c * BL, (c + 1) * BL)
        np.put_along_axis(sum_new[s], perms[c], results[c]["sum_new"], axis=1)
    prev_s = np.asarray(inputs["prev_s"], np.float32)
    prev_s_new = np.concatenate([prev_s, h_new[:, None, :]], 1)
    return (final, h_new, c_new, ct_e, sum_new, prev_s_new)
